# revision 1
# baseline (speedup 1.0000x reference)
"""CrossCueFusion Trainium2 kernel (8 NeuronCores, SPMD via bass/Tile).

Sharding: core c owns output rows [32c, 32c+32) of the [1,64,256,512]
output, feature rows [8c, 8c+8) of the 64x128 feature map (= attention
query positions [1024c, 1024c+1024)). Features are computed 1/8 per
core and AllGather'd so every core has full K / V for the global
attention; scores are computed transposed (S^T[j,i], j on partitions)
so softmax denominators come free from a ones-column in the AV matmul.

Convs use a (dy,ci)-merged contraction (K=96) over 3 row-shifted input
copies, 3 dx taps accumulating in PSUM, and 4 output rows packed via
PE col-tiling (quad layout [128, W]).
"""

import sys

for p in ("/opt/trn_rl_repo", "/opt/trn_rl_repo/concourse"):
    if p not in sys.path:
        sys.path.insert(0, p)

import ml_dtypes
import numpy as np

import concourse.bass as bass
import concourse.mybir as mybir
import concourse.tile as tile
from concourse import bacc
from concourse.bass_utils import run_bass_kernel_spmd

F32 = mybir.dt.float32
BF16 = mybir.dt.bfloat16
BF = ml_dtypes.bfloat16
EXP = mybir.ActivationFunctionType.Exp
ADD = mybir.AluOpType.add
MAX = mybir.AluOpType.max
MULT = mybir.AluOpType.mult

NCORES = 8
H, W = 256, 512
FH, FW = 64, 128  # feature map
HW = FH * FW  # 8192
NJB = HW // 128  # 64 j-blocks
FR = 8  # feature rows per core
ILOC = FR * FW  # 1024 query positions per core
OUTR = 32  # output rows per core
BANDR, BANDW = 37, W + 2  # input band: rows [32c-3, 32c+34), padded width
F1R = 18  # feature conv1 rows per core: abs [16c-1, 16c+17)
R1R = 34  # resid conv1 rows per core: abs [32c-1, 32c+33)
GJB = 3  # score j-blocks per exp group

_CACHE = {}


def _quads(nrows):
    out = []
    for q in range((nrows + 3) // 4):
        out.append((q, min(4, nrows - 4 * q)))
    return out


def _prep(inputs):
    mono = np.asarray(inputs["mono_pseudo_cost"])[0]
    cost = np.asarray(inputs["cost_volume"])[0]
    g = float(np.asarray(inputs["gamma"]).reshape(-1)[0])

    def band(img, c):
        b = np.zeros((32, BANDR, BANDW), np.float32)
        r0 = 32 * c - 3
        lo, hi = max(0, r0), min(H, r0 + BANDR)
        b[:, lo - r0 : hi - r0, 1:513] = img[:, lo:hi, :]
        return b.astype(BF)

    w3k = np.zeros((96, 6, 3, 32), np.float32)
    conv_bias = np.zeros((128, 6), np.float32)
    names = [
        ("me_w1", "me_b1"),
        ("xe_w1", "xe_b1"),
        ("me_w2", "me_b2"),
        ("xe_w2", "xe_b2"),
        ("mr_w1", "mr_b1"),
        ("mr_w2", "mr_b2"),
    ]
    for cv, (wn, bn) in enumerate(names):
        w3 = np.asarray(inputs[wn])  # [o, ci, dy, dx]
        for dy in range(3):
            # [ci, dx, o]
            w3k[32 * dy : 32 * dy + 32, cv] = np.transpose(w3[:, :, dy, :], (1, 2, 0))
        conv_bias[:, cv] = np.tile(np.asarray(inputs[bn]), 4)
    w3k = w3k.reshape(96, 6 * 3 * 32).astype(BF)

    wkq = np.zeros((32, 128), np.float32)
    bias_kq = np.zeros((32, 4), np.float32)
    for br, (kw, kb, qw, qb) in enumerate(
        [("mk_w", "mk_b", "mq_w", "mq_b"), ("xk_w", "xk_b", "xq_w", "xq_b")]
    ):
        wkq[:, br * 64 : br * 64 + 32] = np.asarray(inputs[kw]).T
        wkq[:, br * 64 + 32 : br * 64 + 64] = np.asarray(inputs[qw]).T
        bias_kq[:, br * 2] = np.asarray(inputs[kb])
        bias_kq[:, br * 2 + 1] = np.asarray(inputs[qb])
    wkq = wkq.astype(BF)

    wvT = np.zeros((96, 64), np.float32)
    for br, vw in enumerate(["mv_w", "xv_w"]):
        t = np.asarray(inputs[vw]).T  # [ci, c]
        for rep in range(3):
            wvT[32 * rep : 32 * rep + 32, br * 32 : br * 32 + 32] = t
    wvT = wvT.astype(BF)

    # stream m output (multi_out) uses multi values (xv); stream x uses mv
    bvg = np.stack(
        [g * np.asarray(inputs["xv_b"]), g * np.asarray(inputs["mv_b"])], axis=1
    ).astype(np.float32)  # [32, 2]

    wxr = np.asarray(inputs["xr_w"]).T.astype(BF)  # [ci, o]
    bias_xr = np.tile(np.asarray(inputs["xr_b"]), 4).reshape(128, 1).astype(np.float32)
    gamma_arr = np.full((1, 1), g, np.float32)

    in_maps = []
    for c in range(NCORES):
        masks = np.zeros((128, 14), np.float32)
        for q in range(5):  # feat1 quads
            for j in range(4):
                r = 16 * c - 1 + 4 * q + j
                masks[32 * j : 32 * j + 32, q] = 1.0 if 0 <= r < 128 else 0.0
        for q in range(9):  # resid1 quads
            for j in range(4):
                r = 32 * c - 1 + 4 * q + j
                masks[32 * j : 32 * j + 32, 5 + q] = 1.0 if 0 <= r < H else 0.0
        in_maps.append(
            {
                "band_m": band(mono, c),
                "band_x": band(cost, c),
                "w3k": w3k,
                "conv_bias": conv_bias,
                "masks": masks,
                "wkq": wkq,
                "bias_kq": bias_kq,
                "wvT": wvT,
                "bvg": bvg,
                "wxr": wxr,
                "bias_xr": bias_xr,
                "gamma_in": gamma_arr,
            }
        )
    return in_maps


def build():
    nc = bacc.Bacc(None)
    band_m = nc.declare_dram_parameter("band_m", [32, BANDR, BANDW], BF16, False)
    band_x = nc.declare_dram_parameter("band_x", [32, BANDR, BANDW], BF16, False)
    w3k_d = nc.declare_dram_parameter("w3k", [96, 576], BF16, False)
    conv_bias_d = nc.declare_dram_parameter("conv_bias", [128, 6], F32, False)
    masks_d = nc.declare_dram_parameter("masks", [128, 14], F32, False)
    wkq_d = nc.declare_dram_parameter("wkq", [32, 128], BF16, False)
    bias_kq_d = nc.declare_dram_parameter("bias_kq", [32, 4], F32, False)
    wvT_d = nc.declare_dram_parameter("wvT", [96, 64], BF16, False)
    bvg_d = nc.declare_dram_parameter("bvg", [32, 2], F32, False)
    wxr_d = nc.declare_dram_parameter("wxr", [32, 32], BF16, False)
    bias_xr_d = nc.declare_dram_parameter("bias_xr", [128, 1], F32, False)
    gamma_d = nc.declare_dram_parameter("gamma_in", [1, 1], F32, False)
    out_d = nc.declare_dram_parameter("out", [64, OUTR, W], F32, True)

    with tile.TileContext(nc) as tc:
        _emit(nc, tc, locals())
    nc.finalize()
    return nc


def _emit(nc, tc, d):
    band = {0: d["band_m"], 1: d["band_x"]}
    w3k_d, conv_bias_d, masks_d = d["w3k_d"], d["conv_bias_d"], d["masks_d"]
    wkq_d, bias_kq_d, wvT_d = d["wkq_d"], d["bias_kq_d"], d["wvT_d"]
    bvg_d, wxr_d, bias_xr_d, gamma_d = (
        d["bvg_d"],
        d["wxr_d"],
        d["bias_xr_d"],
        d["gamma_d"],
    )
    out_d = d["out_d"]

    import contextlib

    ctx = contextlib.ExitStack()
    with ctx:
        persist = ctx.enter_context(tc.tile_pool(name="persist", bufs=1))
        dram = ctx.enter_context(tc.tile_pool(name="dram", bufs=1, space="DRAM"))
        psum = ctx.enter_context(tc.tile_pool(name="psum", bufs=1, space="PSUM"))
        small = ctx.enter_context(tc.tile_pool(name="small", bufs=2))

        # ---- persistent tiles ----
        shift3_m = persist.tile([96, 35, BANDW], BF16)
        w3k_sb = persist.tile([96, 576], BF16)
        conv_bias_sb = persist.tile([128, 6], F32)
        masks_sb = persist.tile([128, 14], F32)
        wkq_sb = persist.tile([32, 128], BF16)
        bias_kq_sb = persist.tile([32, 4], F32)
        wvT_sb = persist.tile([96, 64], BF16)
        bvg_sb = persist.tile([32, 2], F32)
        wxr_sb = persist.tile([32, 32], BF16)
        bias_xr_sb = persist.tile([128, 1], F32)
        gamma_sb = persist.tile([1, 1], F32)
        krep = {
            0: persist.tile([96, HW], BF16, name="krep_m"),
            1: persist.tile([96, HW], BF16, name="krep_x"),
        }
        qrep = {
            0: persist.tile([96, ILOC], BF16, name="qrep_m"),
            1: persist.tile([96, ILOC], BF16, name="qrep_x"),
        }
        vt = {
            0: persist.tile([128, NJB * 33], BF16, name="vt_m"),
            1: persist.tile([128, NJB * 33], BF16, name="vt_x"),
        }
        attnrep = {
            0: persist.tile([128, ILOC], F32, name="attnrep_m"),
            1: persist.tile([128, ILOC], F32, name="attnrep_x"),
        }

        for dst, src in [
            (w3k_sb, w3k_d),
            (conv_bias_sb, conv_bias_d),
            (masks_sb, masks_d),
            (wkq_sb, wkq_d),
            (bias_kq_sb, bias_kq_d),
            (wvT_sb, wvT_d),
            (bvg_sb, bvg_d),
            (wxr_sb, wxr_d),
            (bias_xr_sb, bias_xr_d),
            (gamma_sb, gamma_d),
        ]:
            nc.sync.dma_start(out=dst[:], in_=src[:])

        ag_in = {
            0: dram.tile([32, FR, FW], BF16, name="ag_in_m"),
            1: dram.tile([32, FR, FW], BF16, name="ag_in_x"),
        }
        ag_out = {
            0: dram.tile([NCORES, 32, FR, FW], BF16, addr_space="Shared", name="ag_out_m"),
            1: dram.tile([NCORES, 32, FR, FW], BF16, addr_space="Shared", name="ag_out_x"),
        }
        rbounce = dram.tile([4, 512], F32)

        for dy in range(3):
            nc.sync.dma_start(
                out=shift3_m[32 * dy : 32 * dy + 32, :, :],
                in_=band[0][:, dy : dy + 35, :],
            )

        nc.vector.memset(vt[0][:], 1.0)
        nc.vector.memset(vt[1][:], 1.0)

        # =========== Phase A: feature extraction (both branches) ==========
        with tc.tile_pool(name="early", bufs=1) as early:
            shift3_x = early.tile([96, 35, BANDW], BF16)
            for dy in range(3):
                nc.sync.dma_start(
                    out=shift3_x[32 * dy : 32 * dy + 32, :, :],
                    in_=band[1][:, dy : dy + 35, :],
                )
            shift3 = {0: shift3_m, 1: shift3_x}

            for br in range(2):
                cv1, cv2 = br, 2 + br
                feat1q = early.tile([128, 5, 258], BF16, name=f"feat1q_{br}")
                nc.vector.memset(feat1q[:], 0.0)
                s3 = shift3[br]
                for q, jm in _quads(F1R):
                    ps = psum.tile([128, 512], F32, name=f"f1ps_{br}_{q}", tag="convps")
                    for dx in range(3):
                        for j in range(jm):
                            nc.tensor.matmul(
                                ps[32 * j : 32 * j + 32, 0:256],
                                w3k_sb[:, (cv1 * 3 + dx) * 32 : (cv1 * 3 + dx) * 32 + 32],
                                s3[:, 2 * (4 * q + j), dx : dx + 511 : 2],
                                start=(dx == 0),
                                stop=(dx == 2),
                                tile_position=(0, 32 * j),
                            )
                    pm = 32 * jm
                    ev = small.tile([128, 256], F32, tag="ev")
                    nc.vector.tensor_scalar(
                        ev[0:pm, :],
                        ps[0:pm, 0:256],
                        conv_bias_sb[0:pm, cv1 : cv1 + 1],
                        0.0,
                        ADD,
                        MAX,
                    )
                    nc.vector.tensor_scalar(
                        feat1q[0:pm, q, 1:257],
                        ev[0:pm, :],
                        masks_sb[0:pm, q : q + 1],
                        None,
                        MULT,
                    )

                # build conv2 input shifts [96, 17, 258]
                sf2 = early.tile([96, 17, 258], BF16, name=f"sf2_{br}")
                for dy in range(3):
                    for jj in range(4):
                        qs = [
                            q
                            for q, jmq in _quads(F1R)
                            if jj < jmq and dy <= 4 * q + jj < dy + 17
                        ]
                        if not qs:
                            continue
                        q0, q1 = qs[0], qs[-1] + 1
                        r0 = 4 * q0 + jj - dy
                        r1 = r0 + 4 * (q1 - q0 - 1) + 1
                        nc.sync.dma_start(
                            out=sf2[32 * dy : 32 * dy + 32, r0:r1:4, :],
                            in_=feat1q[32 * jj : 32 * jj + 32, q0:q1, :],
                        )

                featloc = early.tile([128, 2, FW], BF16, name=f"featloc_{br}")
                for q, jm in _quads(FR):
                    ps = psum.tile([128, 512], F32, name=f"f2ps_{br}_{q}", tag="convps")
                    for dx in range(3):
                        for j in range(jm):
                            nc.tensor.matmul(
                                ps[32 * j : 32 * j + 32, 0:128],
                                w3k_sb[:, (cv2 * 3 + dx) * 32 : (cv2 * 3 + dx) * 32 + 32],
                                sf2[:, 2 * (4 * q + j), dx : dx + 255 : 2],
                                start=(dx == 0),
                                stop=(dx == 2),
                                tile_position=(0, 32 * j),
                            )
                    nc.vector.tensor_scalar(
                        featloc[:, q, :],
                        ps[:, 0:128],
                        conv_bias_sb[:, cv2 : cv2 + 1],
                        0.0,
                        ADD,
                        MAX,
                    )
                for j in range(4):
                    nc.sync.dma_start(
                        out=ag_in[br][:, j : FR : 4, :],
                        in_=featloc[32 * j : 32 * j + 32, :, :],
                    )
                nc.gpsimd.collective_compute(
                    "AllGather",
                    mybir.AluOpType.bypass,
                    replica_groups=[list(range(NCORES))],
                    ins=[ag_in[br][:]],
                    outs=[ag_out[br][:]],
                )

            # =========== Phase B: projections ==========
            for br in range(2):
                frep = early.tile([96, HW], BF16, name="frep", tag="frep", bufs=1)
                for rep in range(3):
                    src = bass.AP(
                        tensor=ag_out[br].tensor,
                        offset=ag_out[br].offset,
                        ap=[
                            [FR * FW, 32],  # ci
                            [32 * FR * FW, NCORES],  # core
                            [FW, FR],  # r
                            [1, FW],  # x
                        ],
                    )
                    nc.sync.dma_start(out=frep[32 * rep : 32 * rep + 32, :], in_=src)

                # k projection over full hw
                for ch in range(16):
                    ps = psum.tile([128, 512], F32, name=f"kps_{br}_{ch}", tag="convps")
                    nc.tensor.matmul(
                        ps[0:32, :],
                        wkq_sb[:, br * 64 : br * 64 + 32],
                        frep[0:32, 512 * ch : 512 * ch + 512],
                        start=True,
                        stop=True,
                    )
                    nc.vector.tensor_scalar(
                        krep[br][0:32, 512 * ch : 512 * ch + 512],
                        ps[0:32, :],
                        bias_kq_sb[:, br * 2 : br * 2 + 1],
                        None,
                        ADD,
                    )
                for rep in range(1, 3):
                    nc.sync.dma_start(
                        out=krep[br][32 * rep : 32 * rep + 32, :], in_=krep[br][0:32, :]
                    )

                # q projection over local 1024
                qrhs = early.tile([32, ILOC], BF16, name="qrhs", tag="qrhs", bufs=2)
                nc.sync.dma_start(out=qrhs[:], in_=ag_in[br][:])
                for ch in range(2):
                    ps = psum.tile([128, 512], F32, name=f"qps_{br}_{ch}", tag="convps")
                    nc.tensor.matmul(
                        ps[0:32, :],
                        wkq_sb[:, br * 64 + 32 : br * 64 + 64],
                        qrhs[:, 512 * ch : 512 * ch + 512],
                        start=True,
                        stop=True,
                    )
                    nc.vector.tensor_scalar(
                        qrep[br][0:32, 512 * ch : 512 * ch + 512],
                        ps[0:32, :],
                        bias_kq_sb[:, br * 2 + 1 : br * 2 + 2],
                        None,
                        ADD,
                    )
                for rep in range(1, 3):
                    nc.sync.dma_start(
                        out=qrep[br][32 * rep : 32 * rep + 32, :], in_=qrep[br][0:32, :]
                    )

                # V^T blocks [128, 33] per jb (col 32 stays 1.0 from memset)
                vtv = vt[br][:].rearrange("p (b c) -> p b c", c=33)
                for g0 in range(0, NJB, GJB):
                    jbs = list(range(g0, min(g0 + GJB, NJB)))
                    ps = psum.tile(
                        [128, 1536], F32, name=f"vtps_{br}_{g0}", tag="spsum", bufs=2
                    )
                    for t, jb in enumerate(jbs):
                        nc.tensor.matmul(
                            ps[:, 512 * t : 512 * t + 32],
                            frep[32 * t : 32 * t + 32, 128 * jb : 128 * jb + 128],
                            wvT_sb[32 * t : 32 * t + 32, br * 32 : br * 32 + 32],
                            start=True,
                            stop=True,
                            tile_position=(32 * t, 0),
                        )
                    psv = ps[:].rearrange("p (t n) -> p t n", n=512)
                    nc.vector.tensor_copy(
                        vtv[:, jbs[0] : jbs[0] + len(jbs), 0:32], psv[:, 0 : len(jbs), 0:32]
                    )

        # =========== Phase C: attention ==========
        for ich in range(2):
            av = psum.tile([128, 512], F32, name=f"av_{ich}", tag="av", bufs=1)
            for g0 in range(0, NJB, GJB):
                jbs = list(range(g0, min(g0 + GJB, NJB)))
                ex = {}
                for br in range(2):
                    sp = psum.tile(
                        [128, 1536], F32, name=f"sp_{ich}_{g0}_{br}", tag="spsum", bufs=2
                    )
                    for t, jb in enumerate(jbs):
                        nc.tensor.matmul(
                            sp[:, 512 * t : 512 * t + 512],
                            krep[br][32 * t : 32 * t + 32, 128 * jb : 128 * jb + 128],
                            qrep[br][32 * t : 32 * t + 32, 512 * ich : 512 * ich + 512],
                            start=True,
                            stop=True,
                            tile_position=(32 * t, 0),
                        )
                    e = small.tile(
                        [128, 1536], BF16, name=f"ex_{ich}_{g0}_{br}", tag=f"exp{br}",
                        bufs=3,
                    )
                    n = 512 * len(jbs)
                    nc.scalar.activation(e[:, 0:n], sp[:, 0:n], EXP)
                    ex[br] = e
                for t, jb in enumerate(jbs):
                    # stream m (mono scores) x multi values -> out half 0
                    nc.tensor.matmul(
                        av[0:33, :],
                        vt[1][:, 33 * jb : 33 * jb + 33],
                        ex[0][:, 512 * t : 512 * t + 512],
                        start=(jb == 0),
                        stop=(jb == NJB - 1),
                        tile_position=(0, 0),
                        skip_group_check=True,
                    )
                    # stream x (multi scores) x mono values -> out half 1
                    nc.tensor.matmul(
                        av[64:97, :],
                        vt[0][:, 33 * jb : 33 * jb + 33],
                        ex[1][:, 512 * t : 512 * t + 512],
                        start=(jb == 0),
                        stop=(jb == NJB - 1),
                        tile_position=(0, 64),
                        skip_group_check=True,
                    )
            for si in range(2):
                base = 64 * si
                r = small.tile([1, 512], F32, name=f"r_{ich}_{si}", tag="r")
                nc.vector.reciprocal(r[:], av[base + 32 : base + 33, :])
                nc.vector.tensor_scalar(
                    r[:], r[:], gamma_sb[0:1, 0:1], None, MULT
                )
                nc.sync.dma_start(out=rbounce[2 * si + ich, :], in_=r[0:1, :])
                rb = small.tile([32, 512], F32, name=f"rb_{ich}_{si}", tag="rb")
                src = bass.AP(
                    tensor=rbounce.tensor,
                    offset=rbounce.offset + (2 * si + ich) * 512,
                    ap=[[0, 32], [1, 512]],
                )
                nc.sync.dma_start(out=rb[:], in_=src)
                t1 = small.tile([32, 512], F32, name=f"t1_{ich}_{si}", tag="t1")
                nc.vector.tensor_tensor(t1[:], av[base : base + 32, :], rb[:], MULT)
                nc.vector.tensor_scalar(
                    attnrep[si][0:32, 512 * ich : 512 * ich + 512],
                    t1[:],
                    bvg_sb[:, si : si + 1],
                    None,
                    ADD,
                )
                for rep in range(1, 4):
                    nc.sync.dma_start(
                        out=attnrep[si][32 * rep : 32 * rep + 32, 512 * ich : 512 * ich + 512],
                        in_=attnrep[si][0:32, 512 * ich : 512 * ich + 512],
                    )

        # =========== Phase D/E: residuals + assembly ==========
        with tc.tile_pool(name="late", bufs=1) as late:
            resid1q = late.tile([128, 9, BANDW], BF16)
            nc.vector.memset(resid1q[:], 0.0)
            for q, jm in _quads(R1R):
                ps = psum.tile([128, 512], F32, name=f"r1ps_{q}", tag="convps")
                for dx in range(3):
                    for j in range(jm):
                        nc.tensor.matmul(
                            ps[32 * j : 32 * j + 32, :],
                            w3k_sb[:, (4 * 3 + dx) * 32 : (4 * 3 + dx) * 32 + 32],
                            shift3_m[:, 4 * q + j + 1, dx : dx + 512],
                            start=(dx == 0),
                            stop=(dx == 2),
                            tile_position=(0, 32 * j),
                        )
                pm = 32 * jm
                ev = small.tile([128, 512], F32, tag="ev2")
                nc.vector.tensor_scalar(
                    ev[0:pm, :],
                    ps[0:pm, :],
                    conv_bias_sb[0:pm, 4:5],
                    0.0,
                    ADD,
                    MAX,
                )
                nc.vector.tensor_scalar(
                    resid1q[0:pm, q, 1:513],
                    ev[0:pm, :],
                    masks_sb[0:pm, 5 + q : 6 + q],
                    None,
                    MULT,
                )

            sr2 = late.tile([96, 33, BANDW], BF16)
            for dy in range(3):
                for jj in range(4):
                    qs = [
                        q
                        for q, jmq in _quads(R1R)
                        if jj < jmq and dy <= 4 * q + jj < dy + 33
                    ]
                    if not qs:
                        continue
                    q0, q1 = qs[0], qs[-1] + 1
                    r0 = 4 * q0 + jj - dy
                    r1 = r0 + 4 * (q1 - q0 - 1) + 1
                    nc.sync.dma_start(
                        out=sr2[32 * dy : 32 * dy + 32, r0:r1:4, :],
                        in_=resid1q[32 * jj : 32 * jj + 32, q0:q1, :],
                    )

            out_mono = late.tile([128, FR, 512], F32)
            out_multi = late.tile([128, FR, 512], F32)
            for q in range(FR):  # 8 quads of 4 rows each
                # mono residual conv2
                ps = psum.tile([128, 512], F32, name=f"r2ps_{q}", tag="convps")
                for dx in range(3):
                    for j in range(4):
                        nc.tensor.matmul(
                            ps[32 * j : 32 * j + 32, :],
                            w3k_sb[:, (5 * 3 + dx) * 32 : (5 * 3 + dx) * 32 + 32],
                            sr2[:, 4 * q + j, dx : dx + 512],
                            start=(dx == 0),
                            stop=(dx == 2),
                            tile_position=(0, 32 * j),
                        )
                ev = small.tile([128, 512], F32, tag="ev2")
                nc.vector.tensor_scalar(
                    ev[:], ps[:], conv_bias_sb[:, 5:6], 0.0, ADD, MAX
                )
                nc.vector.tensor_tensor(
                    out_mono[:, q, :].rearrange("p (a b) -> p a b", b=4),
                    ev[:].rearrange("p (a b) -> p a b", b=4),
                    attnrep[0][:, 128 * q : 128 * q + 128, None].to_broadcast(
                        [128, 128, 4]
                    ),
                    ADD,
                )
                # multi residual 1x1
                xrhs = late.tile([32, 4, 512], BF16, tag="xrhs", bufs=2)
                nc.sync.dma_start(
                    out=xrhs[:], in_=band[1][:, 4 * q + 3 : 4 * q + 7, 1:513]
                )
                ps2 = psum.tile([128, 512], F32, name=f"xps_{q}", tag="convps")
                for j in range(4):
                    nc.tensor.matmul(
                        ps2[32 * j : 32 * j + 32, :],
                        wxr_sb[:],
                        xrhs[:, j, :],
                        start=True,
                        stop=True,
                        tile_position=(0, 32 * j),
                    )
                ev2 = small.tile([128, 512], F32, tag="ev2")
                nc.vector.tensor_scalar(
                    ev2[:], ps2[:], bias_xr_sb[:, 0:1], 0.0, ADD, MAX
                )
                nc.vector.tensor_tensor(
                    out_multi[:, q, :].rearrange("p (a b) -> p a b", b=4),
                    ev2[:].rearrange("p (a b) -> p a b", b=4),
                    attnrep[1][:, 128 * q : 128 * q + 128, None].to_broadcast(
                        [128, 128, 4]
                    ),
                    ADD,
                )

            for j in range(4):
                nc.sync.dma_start(
                    out=out_d[0:32, j : OUTR : 4, :],
                    in_=out_mono[32 * j : 32 * j + 32, :, :],
                )
                nc.sync.dma_start(
                    out=out_d[32:64, j : OUTR : 4, :],
                    in_=out_multi[32 * j : 32 * j + 32, :, :],
                )


def kernel(**inputs):
    in_maps = _prep(inputs)
    if "nc" not in _CACHE:
        _CACHE["nc"] = build()
    res = run_bass_kernel_spmd(_CACHE["nc"], in_maps, list(range(NCORES)))
    out = np.concatenate([res.results[c]["out"] for c in range(NCORES)], axis=1)
    return out[None].astype(np.float32)



# revision 16
# speedup vs baseline: 1.0244x; 1.0244x over previous
"""CrossCueFusion Trainium2 kernel (8 NeuronCores, SPMD via bass/Tile).

Sharding: core c owns output rows [32c, 32c+32) of the [1,64,256,512]
output, feature rows [8c, 8c+8) of the 64x128 feature map (= attention
query positions [1024c, 1024c+1024)). Features are computed 1/8 per
core and AllGather'd so every core has full K / V for the global
attention; scores are computed transposed (S^T[j,i], j on partitions)
so softmax denominators come free from a ones-column in the AV matmul.

v2 schedule is exp-stream-centric: the softmax exp (16.8M elem/core on
the scalar engine, ~1 elem/lane/cycle) is the hard floor, so branch x
is processed first (its V feeds stream-m AV), projections run right
behind the AllGathers, residual convs run before attention, and the
attention loop pipelines the tensor queue one slot behind the exp
stream (scores g, then AV g-1) so neither engine waits on the other.
"""

import sys

for p in ("/opt/trn_rl_repo", "/opt/trn_rl_repo/concourse"):
    if p not in sys.path:
        sys.path.insert(0, p)

import contextlib

import ml_dtypes
import numpy as np

import concourse.bass as bass
import concourse.mybir as mybir
import concourse.tile as tile
from concourse import bacc
from concourse.bass_utils import run_bass_kernel_spmd

F32 = mybir.dt.float32
BF16 = mybir.dt.bfloat16
BF = ml_dtypes.bfloat16
EXP = mybir.ActivationFunctionType.Exp
RELU = mybir.ActivationFunctionType.Relu
IDENT = mybir.ActivationFunctionType.Identity
COPY = mybir.ActivationFunctionType.Copy
ADD = mybir.AluOpType.add
MAX = mybir.AluOpType.max
MULT = mybir.AluOpType.mult

NCORES = 8
H, W = 256, 512
FH, FW = 64, 128  # feature map
HW = FH * FW  # 8192
NJB = HW // 128  # 64 j-blocks
FR = 8  # feature rows per core
ILOC = FR * FW  # 1024 query positions per core
OUTR = 32  # output rows per core
BANDR, BANDW = 37, W + 2  # input band: rows [32c-3, 32c+34), padded width
F1R = 18  # feature conv1 rows per core: abs [16c-1, 16c+17)
R1R = 34  # resid conv1 rows per core: abs [32c-1, 32c+33)
GJB = 3  # score j-blocks per exp group

_CACHE = {}


def _quads(nrows):
    out = []
    for q in range((nrows + 3) // 4):
        out.append((q, min(4, nrows - 4 * q)))
    return out


def _prep(inputs):
    mono = np.asarray(inputs["mono_pseudo_cost"])[0]
    cost = np.asarray(inputs["cost_volume"])[0]
    g = float(np.asarray(inputs["gamma"]).reshape(-1)[0])

    def band(img, c):
        b = np.zeros((32, BANDR, BANDW), np.float32)
        r0 = 32 * c - 3
        lo, hi = max(0, r0), min(H, r0 + BANDR)
        b[:, lo - r0 : hi - r0, 1:513] = img[:, lo:hi, :]
        return b.astype(BF)

    w3k = np.zeros((96, 6, 3, 32), np.float32)
    conv_bias = np.zeros((128, 6), np.float32)
    names = [
        ("me_w1", "me_b1"),
        ("xe_w1", "xe_b1"),
        ("me_w2", "me_b2"),
        ("xe_w2", "xe_b2"),
        ("mr_w1", "mr_b1"),
        ("mr_w2", "mr_b2"),
    ]
    for cv, (wn, bn) in enumerate(names):
        w3 = np.asarray(inputs[wn])  # [o, ci, dy, dx]
        for dy in range(3):
            # [ci, dx, o]
            w3k[32 * dy : 32 * dy + 32, cv] = np.transpose(w3[:, :, dy, :], (1, 2, 0))
        conv_bias[:, cv] = np.tile(np.asarray(inputs[bn]), 4)
    w3k = w3k.reshape(96, 6 * 3 * 32).astype(BF)

    wkq32 = np.zeros((32, 128), np.float32)
    bias_kq = np.zeros((32, 4), np.float32)
    for br, (kw, kb, qw, qb) in enumerate(
        [("mk_w", "mk_b", "mq_w", "mq_b"), ("xk_w", "xk_b", "xq_w", "xq_b")]
    ):
        wkq32[:, br * 64 : br * 64 + 32] = np.asarray(inputs[kw]).T
        wkq32[:, br * 64 + 32 : br * 64 + 64] = np.asarray(inputs[qw]).T
        bias_kq[:, br * 2] = np.asarray(inputs[kb])
        bias_kq[:, br * 2 + 1] = np.asarray(inputs[qb])
    wkq = np.tile(wkq32, (3, 1)).astype(BF)  # [96, 128]: 3 partition copies

    wvT = np.zeros((96, 64), np.float32)
    for br, vw in enumerate(["mv_w", "xv_w"]):
        t = np.asarray(inputs[vw]).T  # [ci, c]
        for rep in range(3):
            wvT[32 * rep : 32 * rep + 32, br * 32 : br * 32 + 32] = t
    wvT = wvT.astype(BF)

    # stream m output (multi_out) uses multi values (xv); stream x uses mv
    bvg = np.stack(
        [g * np.asarray(inputs["xv_b"]), g * np.asarray(inputs["mv_b"])], axis=1
    ).astype(np.float32)  # [32, 2]

    wxr = np.asarray(inputs["xr_w"]).T.astype(BF)  # [ci, o]
    bias_xr = np.tile(np.asarray(inputs["xr_b"]), 4).reshape(128, 1).astype(np.float32)
    gamma_arr = np.full((128, 1), g, np.float32)

    in_maps = []
    for c in range(NCORES):
        masks = np.zeros((128, 14), np.float32)
        for q in range(5):  # feat1 quads
            for j in range(4):
                r = 16 * c - 1 + 4 * q + j
                masks[32 * j : 32 * j + 32, q] = 1.0 if 0 <= r < 128 else 0.0
        for q in range(9):  # resid1 quads
            for j in range(4):
                r = 32 * c - 1 + 4 * q + j
                masks[32 * j : 32 * j + 32, 5 + q] = 1.0 if 0 <= r < H else 0.0
        in_maps.append(
            {
                "band_m": band(mono, c),
                "band_x": band(cost, c),
                "w3k": w3k,
                "conv_bias": conv_bias,
                "masks": masks,
                "wkq": wkq,
                "bias_kq": bias_kq,
                "wvT": wvT,
                "bvg": bvg,
                "wxr": wxr,
                "bias_xr": bias_xr,
                "gamma_in": gamma_arr,
            }
        )
    return in_maps


def build():
    nc = bacc.Bacc(None)
    band_m = nc.declare_dram_parameter("band_m", [32, BANDR, BANDW], BF16, False)
    band_x = nc.declare_dram_parameter("band_x", [32, BANDR, BANDW], BF16, False)
    w3k_d = nc.declare_dram_parameter("w3k", [96, 576], BF16, False)
    conv_bias_d = nc.declare_dram_parameter("conv_bias", [128, 6], F32, False)
    masks_d = nc.declare_dram_parameter("masks", [128, 14], F32, False)
    wkq_d = nc.declare_dram_parameter("wkq", [96, 128], BF16, False)
    bias_kq_d = nc.declare_dram_parameter("bias_kq", [32, 4], F32, False)
    wvT_d = nc.declare_dram_parameter("wvT", [96, 64], BF16, False)
    bvg_d = nc.declare_dram_parameter("bvg", [32, 2], F32, False)
    wxr_d = nc.declare_dram_parameter("wxr", [32, 32], BF16, False)
    bias_xr_d = nc.declare_dram_parameter("bias_xr", [128, 1], F32, False)
    gamma_d = nc.declare_dram_parameter("gamma_in", [128, 1], F32, False)
    out_d = nc.declare_dram_parameter("out", [64, OUTR, W], F32, True)

    with tile.TileContext(nc) as tc:
        _emit(nc, tc, locals())
    nc.finalize()
    return nc


def _emit(nc, tc, d):
    band = {0: d["band_m"], 1: d["band_x"]}
    w3k_d, conv_bias_d, masks_d = d["w3k_d"], d["conv_bias_d"], d["masks_d"]
    wkq_d, bias_kq_d, wvT_d = d["wkq_d"], d["bias_kq_d"], d["wvT_d"]
    bvg_d, wxr_d, bias_xr_d, gamma_d = (
        d["bvg_d"],
        d["wxr_d"],
        d["bias_xr_d"],
        d["gamma_d"],
    )
    out_d = d["out_d"]

    ctx = contextlib.ExitStack()
    with ctx:
        persist = ctx.enter_context(tc.tile_pool(name="persist", bufs=1))
        dram = ctx.enter_context(tc.tile_pool(name="dram", bufs=1, space="DRAM"))
        psum = ctx.enter_context(tc.tile_pool(name="psum", bufs=1, space="PSUM"))
        small = ctx.enter_context(tc.tile_pool(name="small", bufs=2))

        # PSUM budget (8 banks): sp_m [128,1536] + sp_x [128,1536] + av
        # [128,1024] = 3+3+2. All conv/resid/proj/V^T psum reuses the sp
        # rings via tags so attention needs no extra banks.
        SPTAG = ("spm", "spx")

        def sp_tile(i, name):
            return psum.tile([128, 1536], F32, name=name, tag=SPTAG[i % 2])

        # ---- persistent tiles ----
        w3k_sb = persist.tile([96, 576], BF16)
        conv_bias_sb = persist.tile([128, 6], F32)
        masks_sb = persist.tile([128, 14], F32)
        wkq_sb = persist.tile([96, 128], BF16)
        bias_kq_sb = persist.tile([32, 4], F32)
        wvT_sb = persist.tile([96, 64], BF16)
        bvg_sb = persist.tile([32, 2], F32)
        wxr_sb = persist.tile([32, 32], BF16)
        bias_xr_sb = persist.tile([128, 1], F32)
        gamma_sb = persist.tile([128, 1], F32)
        krep = {
            0: persist.tile([96, HW], BF16, name="krep_m"),
            1: persist.tile([96, HW], BF16, name="krep_x"),
        }
        qrep = {
            0: persist.tile([96, ILOC], BF16, name="qrep_m"),
            1: persist.tile([96, ILOC], BF16, name="qrep_x"),
        }
        vt = {
            0: persist.tile([128, NJB * 33], BF16, name="vt_m"),
            1: persist.tile([128, NJB * 33], BF16, name="vt_x"),
        }
        attnrep = {
            0: persist.tile([128, ILOC], F32, name="attnrep_m"),
            1: persist.tile([128, ILOC], F32, name="attnrep_x"),
        }
        resid_sum = {
            0: persist.tile([128, FR, 512], BF16, name="resid_sum_m"),
            1: persist.tile([128, FR, 512], BF16, name="resid_sum_x"),
        }

        for dst, src in [
            (w3k_sb, w3k_d),
            (conv_bias_sb, conv_bias_d),
            (masks_sb, masks_d),
            (wkq_sb, wkq_d),
            (bias_kq_sb, bias_kq_d),
            (wvT_sb, wvT_d),
            (bvg_sb, bvg_d),
            (wxr_sb, wxr_d),
            (bias_xr_sb, bias_xr_d),
            (gamma_sb, gamma_d),
        ]:
            nc.sync.dma_start(out=dst[:], in_=src[:])

        ag_in = {
            0: dram.tile([32, FR, FW], BF16, name="ag_in_m"),
            1: dram.tile([32, FR, FW], BF16, name="ag_in_x"),
        }
        ag_out = {
            0: dram.tile([NCORES, 32, FR, FW], BF16, addr_space="Shared", name="ag_out_m"),
            1: dram.tile([NCORES, 32, FR, FW], BF16, addr_space="Shared", name="ag_out_x"),
        }
        r1d = dram.tile([2, 1024], F32)
        r2d = dram.tile([2, 1024], F32)

        nc.vector.memset(vt[0][:], 1.0)
        nc.vector.memset(vt[1][:], 1.0)

        # ================= feature extraction =================
        # branch x first: its V (vt[1]) feeds stream-m AV, and the m
        # projections (whose K/Q gate the first exp) then finish last.
        # Big phase buffers are manually-freed single tiles (tc.tile);
        # frees must pop in LIFO order, so resid1q (longest-lived) is
        # allocated first.
        resid1q, free_resid1q = tc.tile([128, 9, BANDW], BF16, name="resid1q")
        nc.vector.memset(resid1q[:], 0.0)
        shift3_m, free_shift3_m = tc.tile([96, 35, BANDW], BF16, name="shift3_m")
        shift3_x, free_shift3_x = tc.tile([96, 35, BANDW], BF16, name="shift3_x")

        def emit_shift3(s3, br):
            # 2 row-chunks per dy so conv1 can start on the first chunk
            for dy in range(3):
                for r0, r1 in ((0, 18), (18, 35)):
                    nc.sync.dma_start(
                        out=s3[32 * dy : 32 * dy + 32, r0:r1, :],
                        in_=band[br][:, dy + r0 : dy + r1, :],
                    )

        def conv_branch(br, s3, spb):
            cv1, cv2 = br, 2 + br
            feat1q, free_feat1q = tc.tile([128, 5, 258], BF16, name=f"feat1q_{br}")
            nc.vector.memset(feat1q[:], 0.0)
            for q, jm in _quads(F1R):
                ps = sp_tile(spb + q, f"f1ps_{br}_{q}")
                for dx in range(3):
                    for j in range(jm):
                        nc.tensor.matmul(
                            ps[32 * j : 32 * j + 32, 0:256],
                            w3k_sb[:, (cv1 * 3 + dx) * 32 : (cv1 * 3 + dx) * 32 + 32],
                            s3[:, 2 * (4 * q + j), dx : dx + 511 : 2],
                            start=(dx == 0),
                            stop=(dx == 2),
                            tile_position=(0, 32 * j),
                        )
                pm = 32 * jm
                ev = small.tile([128, 256], F32, tag="ev")
                nc.scalar.activation(
                    ev[0:pm, :], ps[0:pm, 0:256], RELU,
                    bias=conv_bias_sb[0:pm, cv1 : cv1 + 1],
                )
                nc.scalar.activation(
                    feat1q[0:pm, q, 1:257], ev[0:pm, :], COPY,
                    scale=masks_sb[0:pm, q : q + 1],
                )

            # conv2 input shifts [96, 17, 258]
            sf2, free_sf2 = tc.tile([96, 17, 258], BF16, name=f"sf2_{br}")
            for dy in range(3):
                for jj in range(4):
                    qs = [
                        q
                        for q, jmq in _quads(F1R)
                        if jj < jmq and dy <= 4 * q + jj < dy + 17
                    ]
                    if not qs:
                        continue
                    q0, q1 = qs[0], qs[-1] + 1
                    r0 = 4 * q0 + jj - dy
                    r1 = r0 + 4 * (q1 - q0 - 1) + 1
                    nc.sync.dma_start(
                        out=sf2[32 * dy : 32 * dy + 32, r0:r1:4, :],
                        in_=feat1q[32 * jj : 32 * jj + 32, q0:q1, :],
                    )

            featloc, free_featloc = tc.tile([128, 2, FW], BF16, name=f"featloc_{br}")
            for q, jm in _quads(FR):
                ps = sp_tile(spb + 5 + q, f"f2ps_{br}_{q}")
                for dx in range(3):
                    for j in range(jm):
                        nc.tensor.matmul(
                            ps[32 * j : 32 * j + 32, 0:128],
                            w3k_sb[:, (cv2 * 3 + dx) * 32 : (cv2 * 3 + dx) * 32 + 32],
                            sf2[:, 2 * (4 * q + j), dx : dx + 255 : 2],
                            start=(dx == 0),
                            stop=(dx == 2),
                            tile_position=(0, 32 * j),
                        )
                nc.scalar.activation(
                    featloc[:, q, :], ps[:, 0:128], RELU,
                    bias=conv_bias_sb[:, cv2 : cv2 + 1],
                )
            for j in range(4):
                nc.sync.dma_start(
                    out=ag_in[br][:, j : FR : 4, :],
                    in_=featloc[32 * j : 32 * j + 32, :, :],
                )
            nc.gpsimd.collective_compute(
                "AllGather",
                mybir.AluOpType.bypass,
                replica_groups=[list(range(NCORES))],
                ins=[ag_in[br][:]],
                outs=[ag_out[br][:]],
            )
            free_featloc()
            free_sf2()
            free_feat1q()

        def proj_branch(br, spb):
            # gather full features [96, HW] (3 partition copies)
            frep, free_frep = tc.tile([96, HW], BF16, name=f"frep_{br}")
            for rep in range(3):
                src = bass.AP(
                    tensor=ag_out[br].tensor,
                    offset=ag_out[br].offset,
                    ap=[
                        [FR * FW, 32],  # ci
                        [32 * FR * FW, NCORES],  # core
                        [FW, FR],  # r
                        [1, FW],  # x
                    ],
                )
                nc.sync.dma_start(out=frep[32 * rep : 32 * rep + 32, :], in_=src)

            # k projection over full hw, 3-packed over ch chunks
            nch = 0
            rnd = 0
            while nch < 16:
                take = min(3, 16 - nch)
                ps = sp_tile(spb + rnd, f"kps_{br}_{rnd}")
                for t in range(take):
                    ch = nch + t
                    nc.tensor.matmul(
                        ps[0:32, 512 * t : 512 * t + 512],
                        wkq_sb[32 * t : 32 * t + 32, br * 64 : br * 64 + 32],
                        frep[32 * t : 32 * t + 32, 512 * ch : 512 * ch + 512],
                        start=True,
                        stop=True,
                        tile_position=(32 * t, 0),
                    )
                n = 512 * take
                nc.scalar.activation(
                    krep[br][0:32, 512 * nch : 512 * nch + n], ps[0:32, 0:n], IDENT,
                    bias=bias_kq_sb[:, br * 2 : br * 2 + 1],
                )
                nch += take
                rnd += 1
            # replicate K to partition copies, chunked for early scores
            for rep in range(1, 3):
                for c0 in range(0, HW, 2048):
                    nc.sync.dma_start(
                        out=krep[br][32 * rep : 32 * rep + 32, c0 : c0 + 2048],
                        in_=krep[br][0:32, c0 : c0 + 2048],
                    )

            # q projection over local 1024
            qrhs = small.tile([32, ILOC], BF16, name="qrhs", tag="qrhs", bufs=2)
            nc.sync.dma_start(out=qrhs[:], in_=ag_in[br][:])
            ps = sp_tile(spb + rnd, f"qps_{br}")
            for t in range(2):
                nc.tensor.matmul(
                    ps[0:32, 512 * t : 512 * t + 512],
                    wkq_sb[0:32, br * 64 + 32 : br * 64 + 64],
                    qrhs[:, 512 * t : 512 * t + 512],
                    start=True,
                    stop=True,
                    tile_position=(0, 0),
                )
            nc.scalar.activation(
                qrep[br][0:32, :], ps[0:32, 0:1024], IDENT,
                bias=bias_kq_sb[:, br * 2 + 1 : br * 2 + 2],
            )
            for rep in range(1, 3):
                nc.sync.dma_start(
                    out=qrep[br][32 * rep : 32 * rep + 32, :], in_=qrep[br][0:32, :]
                )

            # V^T blocks [128, 33] per jb (col 32 stays 1.0 from memset)
            vtv = vt[br][:].rearrange("p (b c) -> p b c", c=33)
            for gi, g0 in enumerate(range(0, NJB, GJB)):
                jbs = list(range(g0, min(g0 + GJB, NJB)))
                ps = sp_tile(spb + rnd + 1 + gi, f"vtps_{br}_{g0}")
                for t, jb in enumerate(jbs):
                    nc.tensor.matmul(
                        ps[:, 512 * t : 512 * t + 32],
                        frep[32 * t : 32 * t + 32, 128 * jb : 128 * jb + 128],
                        wvT_sb[32 * t : 32 * t + 32, br * 32 : br * 32 + 32],
                        start=True,
                        stop=True,
                        tile_position=(32 * t, 0),
                    )
                psv = ps[:].rearrange("p (t n) -> p t n", n=512)
                nc.vector.tensor_copy(
                    vtv[:, jbs[0] : jbs[0] + len(jbs), 0:32], psv[:, 0 : len(jbs), 0:32]
                )
            free_frep()

        emit_shift3(shift3_x, 1)
        emit_shift3(shift3_m, 0)
        conv_branch(1, shift3_x, 0)
        free_shift3_x()
        conv_branch(0, shift3_m, 1)

        # ================= residual conv1 (needs shift3_m) =================
        if True:
            for q, jm in _quads(R1R):
                ps = sp_tile(q, f"r1ps_{q}")
                for dx in range(3):
                    for j in range(jm):
                        nc.tensor.matmul(
                            ps[32 * j : 32 * j + 32, 0:512],
                            w3k_sb[:, (4 * 3 + dx) * 32 : (4 * 3 + dx) * 32 + 32],
                            shift3_m[:, 4 * q + j + 1, dx : dx + 512],
                            start=(dx == 0),
                            stop=(dx == 2),
                            tile_position=(0, 32 * j),
                        )
                pm = 32 * jm
                ev = small.tile([128, 512], F32, tag="ev2")
                nc.scalar.activation(
                    ev[0:pm, :], ps[0:pm, 0:512], RELU,
                    bias=conv_bias_sb[0:pm, 4:5],
                )
                nc.scalar.activation(
                    resid1q[0:pm, q, 1:513], ev[0:pm, :], COPY,
                    scale=masks_sb[0:pm, 5 + q : 6 + q],
                )

            free_shift3_m()
            sr2, free_sr2 = tc.tile([96, 33, BANDW], BF16, name="sr2")
            for dy in range(3):
                for jj in range(4):
                    qs = [
                        q
                        for q, jmq in _quads(R1R)
                        if jj < jmq and dy <= 4 * q + jj < dy + 33
                    ]
                    if not qs:
                        continue
                    q0, q1 = qs[0], qs[-1] + 1
                    r0 = 4 * q0 + jj - dy
                    r1 = r0 + 4 * (q1 - q0 - 1) + 1
                    nc.sync.dma_start(
                        out=sr2[32 * dy : 32 * dy + 32, r0:r1:4, :],
                        in_=resid1q[32 * jj : 32 * jj + 32, q0:q1, :],
                    )

            # ---- projections x (needs AG x; runs while resid DMAs fly) ----
            proj_branch(1, 9)

            # ---- resid conv2 -> resid_sum[0]; xr 1x1 -> resid_sum[1] ----
            for q in range(FR):
                ps = sp_tile(q, f"r2ps_{q}")
                for dx in range(3):
                    for j in range(4):
                        nc.tensor.matmul(
                            ps[32 * j : 32 * j + 32, 0:512],
                            w3k_sb[:, (5 * 3 + dx) * 32 : (5 * 3 + dx) * 32 + 32],
                            sr2[:, 4 * q + j, dx : dx + 512],
                            start=(dx == 0),
                            stop=(dx == 2),
                            tile_position=(0, 32 * j),
                        )
                nc.scalar.activation(
                    resid_sum[0][:, q, :], ps[:, 0:512], RELU,
                    bias=conv_bias_sb[:, 5:6],
                )
                xrhs = small.tile([32, 4, 512], BF16, tag="xrhs", bufs=2)
                nc.sync.dma_start(
                    out=xrhs[:], in_=band[1][:, 4 * q + 3 : 4 * q + 7, 1:513]
                )
                ps2 = sp_tile(q + 1, f"xps_{q}")
                for j in range(4):
                    nc.tensor.matmul(
                        ps2[32 * j : 32 * j + 32, 0:512],
                        wxr_sb[:],
                        xrhs[:, j, :],
                        start=True,
                        stop=True,
                        tile_position=(0, 32 * j),
                    )
                nc.scalar.activation(
                    resid_sum[1][:, q, :], ps2[:, 0:512], RELU,
                    bias=bias_xr_sb[:, 0:1],
                )
            free_sr2()
            free_resid1q()

        # ---- projections m (needs AG m) ----
        proj_branch(0, 0)

        # ================= attention =================
        # av accumulators: rows 0-32 stream m (mono scores x multi V),
        # rows 64-96 stream x; cols 512*ich. Row 32/96 hold softmax
        # denominators via the ones column in vt.
        av = psum.tile([128, 1024], F32, name="av", tag="av")
        groups = []
        for g0 in range(0, NJB, GJB):
            jbs = list(range(g0, min(g0 + GJB, NJB)))
            for ich in range(2):
                groups.append((g0, ich, jbs))

        def emit_scores(slot):
            g0, ich, jbs = groups[slot]
            ex = {}
            for br in range(2):
                sp = psum.tile(
                    [128, 1536], F32, name=f"sp_{br}_{g0}_{ich}", tag=SPTAG[br]
                )
                for t, jb in enumerate(jbs):
                    nc.tensor.matmul(
                        sp[:, 512 * t : 512 * t + 512],
                        krep[br][32 * t : 32 * t + 32, 128 * jb : 128 * jb + 128],
                        qrep[br][32 * t : 32 * t + 32, 512 * ich : 512 * ich + 512],
                        start=True,
                        stop=True,
                        tile_position=(32 * t, 0),
                    )
                e = small.tile(
                    [128, 1536], BF16, name=f"ex_{br}_{g0}_{ich}", tag=f"exp{br}",
                    bufs=3,
                )
                n = 512 * len(jbs)
                nc.scalar.activation(e[:, 0:n], sp[:, 0:n], EXP)
                ex[br] = e
            return ex

        def emit_av(slot, ex):
            g0, ich, jbs = groups[slot]
            for t, jb in enumerate(jbs):
                # stream m (mono scores) x multi values -> out half 0
                nc.tensor.matmul(
                    av[0:33, 512 * ich : 512 * ich + 512],
                    vt[1][:, 33 * jb : 33 * jb + 33],
                    ex[0][:, 512 * t : 512 * t + 512],
                    start=(jb == 0),
                    stop=(jb == NJB - 1),
                    tile_position=(0, 0),
                    skip_group_check=True,
                )
                # stream x (multi scores) x mono values -> out half 1
                nc.tensor.matmul(
                    av[64:97, 512 * ich : 512 * ich + 512],
                    vt[0][:, 33 * jb : 33 * jb + 33],
                    ex[1][:, 512 * t : 512 * t + 512],
                    start=(jb == 0),
                    stop=(jb == NJB - 1),
                    tile_position=(0, 64),
                    skip_group_check=True,
                )

        pending = None
        for slot in range(len(groups)):
            ex = emit_scores(slot)
            if pending is not None:
                emit_av(*pending)
            pending = (slot, ex)
        emit_av(*pending)

        # ================= tail =================
        tailp = ctx.enter_context(tc.tile_pool(name="tailp", bufs=1))
        # denominators: av rows 32 (stream m) and 96 (stream x), 1024 wide.
        den_sb = tailp.tile([128, 1024], F32, name="den_sb")
        nc.scalar.copy(den_sb[32:33, :], av[32:33, :])
        nc.scalar.copy(den_sb[96:97, :], av[96:97, :])
        nc.sync.dma_start(out=r1d[0, :], in_=den_sb[32:33, :])
        nc.sync.dma_start(out=r1d[1, :], in_=den_sb[96:97, :])
        rcp = tailp.tile([128, 16], F32, name="rcp")
        src = bass.AP(tensor=r1d.tensor, offset=r1d.offset, ap=[[16, 128], [1, 16]])
        nc.sync.dma_start(out=rcp[:], in_=src)
        nc.vector.reciprocal(rcp[:], rcp[:])
        nc.vector.tensor_scalar(rcp[:], rcp[:], gamma_sb[:, 0:1], None, MULT)
        dst = bass.AP(tensor=r2d.tensor, offset=r2d.offset, ap=[[16, 128], [1, 16]])
        nc.sync.dma_start(out=dst, in_=rcp[:])

        for si in range(2):
            base = 64 * si
            rb = tailp.tile([32, ILOC], F32, name=f"rb_{si}", tag="rb", bufs=2)
            src = bass.AP(
                tensor=r2d.tensor,
                offset=r2d.offset + si * 1024,
                ap=[[0, 32], [1, 1024]],
            )
            nc.sync.dma_start(out=rb[:], in_=src)
            t1 = tailp.tile([32, ILOC], F32, name=f"t1_{si}", tag="t1", bufs=2)
            nc.vector.tensor_tensor(t1[:], av[base : base + 32, :], rb[:], MULT)
            nc.scalar.activation(
                attnrep[si][0:32, :], t1[:], IDENT, bias=bvg_sb[:, si : si + 1]
            )
            for rep in range(1, 4):
                nc.sync.dma_start(
                    out=attnrep[si][32 * rep : 32 * rep + 32, :],
                    in_=attnrep[si][0:32, :],
                )

        # final adds into f32 staging (2 quads per chunk), then stream out
        for si in range(2):
            for c in range(FR // 2):
                outst = tailp.tile(
                    [128, 2, 512], F32, name=f"outst_{si}_{c}", tag="outst", bufs=2
                )
                for k in range(2):
                    q = 2 * c + k
                    nc.vector.tensor_tensor(
                        outst[:, k, :].rearrange("p (a b) -> p a b", b=4),
                        resid_sum[si][:, q, :].rearrange("p (a b) -> p a b", b=4),
                        attnrep[si][:, 128 * q : 128 * q + 128, None].to_broadcast(
                            [128, 128, 4]
                        ),
                        ADD,
                    )
                for j in range(4):
                    r0 = 8 * c + j
                    nc.sync.dma_start(
                        out=out_d[32 * si : 32 * si + 32, r0 : r0 + 5 : 4, :],
                        in_=outst[32 * j : 32 * j + 32, :, :],
                    )


def kernel(**inputs):
    in_maps = _prep(inputs)
    if "nc" not in _CACHE:
        _CACHE["nc"] = build()
    res = run_bass_kernel_spmd(_CACHE["nc"], in_maps, list(range(NCORES)))
    out = np.concatenate([res.results[c]["out"] for c in range(NCORES)], axis=1)
    return out[None].astype(np.float32)


# revision 18
# speedup vs baseline: 1.0290x; 1.0045x over previous
"""CrossCueFusion Trainium2 kernel (8 NeuronCores, SPMD via bass/Tile).

Sharding: core c owns output rows [32c, 32c+32) of the [1,64,256,512]
output, feature rows [8c, 8c+8) of the 64x128 feature map (= attention
query positions [1024c, 1024c+1024)). Features are computed 1/8 per
core and AllGather'd so every core has full K / V for the global
attention; scores are computed transposed (S^T[j,i], j on partitions)
so softmax denominators come free from a ones-column in the AV matmul.

v2 schedule is exp-stream-centric: the softmax exp (16.8M elem/core on
the scalar engine, ~1 elem/lane/cycle) is the hard floor, so branch x
is processed first (its V feeds stream-m AV), projections run right
behind the AllGathers, residual convs run before attention, and the
attention loop pipelines the tensor queue one slot behind the exp
stream (scores g, then AV g-1) so neither engine waits on the other.
"""

import sys

for p in ("/opt/trn_rl_repo", "/opt/trn_rl_repo/concourse"):
    if p not in sys.path:
        sys.path.insert(0, p)

import contextlib

import ml_dtypes
import numpy as np

import concourse.bass as bass
import concourse.mybir as mybir
import concourse.tile as tile
from concourse import bacc
from concourse.bass_utils import run_bass_kernel_spmd

F32 = mybir.dt.float32
BF16 = mybir.dt.bfloat16
BF = ml_dtypes.bfloat16
EXP = mybir.ActivationFunctionType.Exp
RELU = mybir.ActivationFunctionType.Relu
IDENT = mybir.ActivationFunctionType.Identity
COPY = mybir.ActivationFunctionType.Copy
ADD = mybir.AluOpType.add
MAX = mybir.AluOpType.max
MULT = mybir.AluOpType.mult

NCORES = 8
H, W = 256, 512
FH, FW = 64, 128  # feature map
HW = FH * FW  # 8192
NJB = HW // 128  # 64 j-blocks
FR = 8  # feature rows per core
ILOC = FR * FW  # 1024 query positions per core
OUTR = 32  # output rows per core
BANDR, BANDW = 37, W + 2  # input band: rows [32c-3, 32c+34), padded width
F1R = 18  # feature conv1 rows per core: abs [16c-1, 16c+17)
R1R = 34  # resid conv1 rows per core: abs [32c-1, 32c+33)
GJB = 3  # score j-blocks per exp group

_CACHE = {}


def _quads(nrows):
    out = []
    for q in range((nrows + 3) // 4):
        out.append((q, min(4, nrows - 4 * q)))
    return out


def _prep(inputs):
    mono = np.asarray(inputs["mono_pseudo_cost"])[0]
    cost = np.asarray(inputs["cost_volume"])[0]
    g = float(np.asarray(inputs["gamma"]).reshape(-1)[0])

    def band(img, c):
        b = np.zeros((32, BANDR, BANDW), np.float32)
        r0 = 32 * c - 3
        lo, hi = max(0, r0), min(H, r0 + BANDR)
        b[:, lo - r0 : hi - r0, 1:513] = img[:, lo:hi, :]
        return b.astype(BF)

    w3k = np.zeros((96, 6, 3, 32), np.float32)
    conv_bias = np.zeros((128, 6), np.float32)
    names = [
        ("me_w1", "me_b1"),
        ("xe_w1", "xe_b1"),
        ("me_w2", "me_b2"),
        ("xe_w2", "xe_b2"),
        ("mr_w1", "mr_b1"),
        ("mr_w2", "mr_b2"),
    ]
    for cv, (wn, bn) in enumerate(names):
        w3 = np.asarray(inputs[wn])  # [o, ci, dy, dx]
        for dy in range(3):
            # [ci, dx, o]
            w3k[32 * dy : 32 * dy + 32, cv] = np.transpose(w3[:, :, dy, :], (1, 2, 0))
        conv_bias[:, cv] = np.tile(np.asarray(inputs[bn]), 4)
    w3k = w3k.reshape(96, 6 * 3 * 32).astype(BF)

    wkq32 = np.zeros((32, 128), np.float32)
    bias_kq = np.zeros((32, 4), np.float32)
    for br, (kw, kb, qw, qb) in enumerate(
        [("mk_w", "mk_b", "mq_w", "mq_b"), ("xk_w", "xk_b", "xq_w", "xq_b")]
    ):
        wkq32[:, br * 64 : br * 64 + 32] = np.asarray(inputs[kw]).T
        wkq32[:, br * 64 + 32 : br * 64 + 64] = np.asarray(inputs[qw]).T
        bias_kq[:, br * 2] = np.asarray(inputs[kb])
        bias_kq[:, br * 2 + 1] = np.asarray(inputs[qb])
    wkq = np.tile(wkq32, (3, 1)).astype(BF)  # [96, 128]: 3 partition copies

    wvT = np.zeros((96, 64), np.float32)
    for br, vw in enumerate(["mv_w", "xv_w"]):
        t = np.asarray(inputs[vw]).T  # [ci, c]
        for rep in range(3):
            wvT[32 * rep : 32 * rep + 32, br * 32 : br * 32 + 32] = t
    wvT = wvT.astype(BF)

    # stream m output (multi_out) uses multi values (xv); stream x uses mv
    bvg = np.stack(
        [g * np.asarray(inputs["xv_b"]), g * np.asarray(inputs["mv_b"])], axis=1
    ).astype(np.float32)  # [32, 2]

    wxr = np.asarray(inputs["xr_w"]).T.astype(BF)  # [ci, o]
    bias_xr = np.tile(np.asarray(inputs["xr_b"]), 4).reshape(128, 1).astype(np.float32)
    gamma_arr = np.full((128, 1), g, np.float32)

    in_maps = []
    for c in range(NCORES):
        masks = np.zeros((128, 14), np.float32)
        for q in range(5):  # feat1 quads
            for j in range(4):
                r = 16 * c - 1 + 4 * q + j
                masks[32 * j : 32 * j + 32, q] = 1.0 if 0 <= r < 128 else 0.0
        for q in range(9):  # resid1 quads
            for j in range(4):
                r = 32 * c - 1 + 4 * q + j
                masks[32 * j : 32 * j + 32, 5 + q] = 1.0 if 0 <= r < H else 0.0
        in_maps.append(
            {
                "band_m": band(mono, c),
                "band_x": band(cost, c),
                "w3k": w3k,
                "conv_bias": conv_bias,
                "masks": masks,
                "wkq": wkq,
                "bias_kq": bias_kq,
                "wvT": wvT,
                "bvg": bvg,
                "wxr": wxr,
                "bias_xr": bias_xr,
                "gamma_in": gamma_arr,
            }
        )
    return in_maps


def build():
    nc = bacc.Bacc(None)
    band_m = nc.declare_dram_parameter("band_m", [32, BANDR, BANDW], BF16, False)
    band_x = nc.declare_dram_parameter("band_x", [32, BANDR, BANDW], BF16, False)
    w3k_d = nc.declare_dram_parameter("w3k", [96, 576], BF16, False)
    conv_bias_d = nc.declare_dram_parameter("conv_bias", [128, 6], F32, False)
    masks_d = nc.declare_dram_parameter("masks", [128, 14], F32, False)
    wkq_d = nc.declare_dram_parameter("wkq", [96, 128], BF16, False)
    bias_kq_d = nc.declare_dram_parameter("bias_kq", [32, 4], F32, False)
    wvT_d = nc.declare_dram_parameter("wvT", [96, 64], BF16, False)
    bvg_d = nc.declare_dram_parameter("bvg", [32, 2], F32, False)
    wxr_d = nc.declare_dram_parameter("wxr", [32, 32], BF16, False)
    bias_xr_d = nc.declare_dram_parameter("bias_xr", [128, 1], F32, False)
    gamma_d = nc.declare_dram_parameter("gamma_in", [128, 1], F32, False)
    out_d = nc.declare_dram_parameter("out", [64, OUTR, W], F32, True)

    with tile.TileContext(nc) as tc:
        _emit(nc, tc, locals())
    nc.finalize()
    return nc


def _emit(nc, tc, d):
    band = {0: d["band_m"], 1: d["band_x"]}
    w3k_d, conv_bias_d, masks_d = d["w3k_d"], d["conv_bias_d"], d["masks_d"]
    wkq_d, bias_kq_d, wvT_d = d["wkq_d"], d["bias_kq_d"], d["wvT_d"]
    bvg_d, wxr_d, bias_xr_d, gamma_d = (
        d["bvg_d"],
        d["wxr_d"],
        d["bias_xr_d"],
        d["gamma_d"],
    )
    out_d = d["out_d"]

    ctx = contextlib.ExitStack()
    with ctx:
        persist = ctx.enter_context(tc.tile_pool(name="persist", bufs=1))
        dram = ctx.enter_context(tc.tile_pool(name="dram", bufs=1, space="DRAM"))
        psum = ctx.enter_context(tc.tile_pool(name="psum", bufs=1, space="PSUM"))
        small = ctx.enter_context(tc.tile_pool(name="small", bufs=2))

        # PSUM budget (8 banks): sp_m [128,1536] + sp_x [128,1536] + av
        # [128,1024] = 3+3+2. All conv/resid/proj/V^T psum reuses the sp
        # rings via tags so attention needs no extra banks.
        SPTAG = ("spm", "spx")

        def sp_tile(i, name):
            return psum.tile([128, 1536], F32, name=name, tag=SPTAG[i % 2])

        # ---- persistent tiles ----
        w3k_sb = persist.tile([96, 576], BF16)
        conv_bias_sb = persist.tile([128, 6], F32)
        masks_sb = persist.tile([128, 14], F32)
        wkq_sb = persist.tile([96, 128], BF16)
        bias_kq_sb = persist.tile([32, 4], F32)
        wvT_sb = persist.tile([96, 64], BF16)
        bvg_sb = persist.tile([32, 2], F32)
        wxr_sb = persist.tile([32, 32], BF16)
        bias_xr_sb = persist.tile([128, 1], F32)
        gamma_sb = persist.tile([128, 1], F32)
        krep = {
            0: persist.tile([96, HW], BF16, name="krep_m"),
            1: persist.tile([96, HW], BF16, name="krep_x"),
        }
        qrep = {
            0: persist.tile([96, ILOC], BF16, name="qrep_m"),
            1: persist.tile([96, ILOC], BF16, name="qrep_x"),
        }
        vt = {
            0: persist.tile([128, NJB * 33], BF16, name="vt_m"),
            1: persist.tile([128, NJB * 33], BF16, name="vt_x"),
        }
        attnrep = {
            0: persist.tile([128, ILOC], F32, name="attnrep_m"),
            1: persist.tile([128, ILOC], F32, name="attnrep_x"),
        }
        resid_sum = {
            0: persist.tile([128, FR, 512], BF16, name="resid_sum_m"),
            1: persist.tile([128, FR, 512], BF16, name="resid_sum_x"),
        }

        for dst, src in [
            (w3k_sb, w3k_d),
            (conv_bias_sb, conv_bias_d),
            (masks_sb, masks_d),
            (wkq_sb, wkq_d),
            (bias_kq_sb, bias_kq_d),
            (wvT_sb, wvT_d),
            (bvg_sb, bvg_d),
            (wxr_sb, wxr_d),
            (bias_xr_sb, bias_xr_d),
            (gamma_sb, gamma_d),
        ]:
            nc.sync.dma_start(out=dst[:], in_=src[:])

        ag_in = {
            0: dram.tile([32, FR, FW], BF16, name="ag_in_m"),
            1: dram.tile([32, FR, FW], BF16, name="ag_in_x"),
        }
        ag_out = {
            0: dram.tile([NCORES, 32, FR, FW], BF16, addr_space="Shared", name="ag_out_m"),
            1: dram.tile([NCORES, 32, FR, FW], BF16, addr_space="Shared", name="ag_out_x"),
        }
        r1d = dram.tile([2, 1024], F32)
        r2d = dram.tile([2, 1024], F32)

        nc.vector.memset(vt[0][:], 1.0)
        nc.vector.memset(vt[1][:], 1.0)

        # ================= feature extraction =================
        # branch x first: its V (vt[1]) feeds stream-m AV, and the m
        # projections (whose K/Q gate the first exp) then finish last.
        # Big phase buffers are manually-freed single tiles (tc.tile);
        # frees must pop in LIFO order, so resid1q (longest-lived) is
        # allocated first.
        resid1q, free_resid1q = tc.tile([128, 9, BANDW], BF16, name="resid1q")
        nc.vector.memset(resid1q[:], 0.0)
        shift3_m, free_shift3_m = tc.tile([96, 35, BANDW], BF16, name="shift3_m")
        shift3_x, free_shift3_x = tc.tile([96, 35, BANDW], BF16, name="shift3_x")

        def emit_shift3(s3, br, eng):
            # 2 row-chunks per dy so conv1 can start on the first chunk
            for dy in range(3):
                for r0, r1 in ((0, 18), (18, 35)):
                    eng.dma_start(
                        out=s3[32 * dy : 32 * dy + 32, r0:r1, :],
                        in_=band[br][:, dy + r0 : dy + r1, :],
                    )

        def conv_branch(br, s3, spb):
            cv1, cv2 = br, 2 + br
            feat1q, free_feat1q = tc.tile([128, 5, 258], BF16, name=f"feat1q_{br}")
            nc.vector.memset(feat1q[:], 0.0)
            for q, jm in _quads(F1R):
                ps = sp_tile(spb + q, f"f1ps_{br}_{q}")
                for dx in range(3):
                    for j in range(jm):
                        nc.tensor.matmul(
                            ps[32 * j : 32 * j + 32, 0:256],
                            w3k_sb[:, (cv1 * 3 + dx) * 32 : (cv1 * 3 + dx) * 32 + 32],
                            s3[:, 2 * (4 * q + j), dx : dx + 511 : 2],
                            start=(dx == 0),
                            stop=(dx == 2),
                            tile_position=(0, 32 * j),
                        )
                pm = 32 * jm
                ev = small.tile([128, 256], F32, tag="ev")
                nc.vector.tensor_scalar(
                    ev[0:pm, :], ps[0:pm, 0:256],
                    conv_bias_sb[0:pm, cv1 : cv1 + 1], 0.0, ADD, MAX,
                )
                nc.vector.tensor_scalar(
                    feat1q[0:pm, q, 1:257], ev[0:pm, :],
                    masks_sb[0:pm, q : q + 1], None, MULT,
                )

            # conv2 input shifts [96, 17, 258]
            sf2, free_sf2 = tc.tile([96, 17, 258], BF16, name=f"sf2_{br}")
            for dy in range(3):
                for jj in range(4):
                    qs = [
                        q
                        for q, jmq in _quads(F1R)
                        if jj < jmq and dy <= 4 * q + jj < dy + 17
                    ]
                    if not qs:
                        continue
                    q0, q1 = qs[0], qs[-1] + 1
                    r0 = 4 * q0 + jj - dy
                    r1 = r0 + 4 * (q1 - q0 - 1) + 1
                    nc.gpsimd.dma_start(
                        out=sf2[32 * dy : 32 * dy + 32, r0:r1:4, :],
                        in_=feat1q[32 * jj : 32 * jj + 32, q0:q1, :],
                    )

            featloc, free_featloc = tc.tile([128, 2, FW], BF16, name=f"featloc_{br}")
            for q, jm in _quads(FR):
                ps = sp_tile(spb + 5 + q, f"f2ps_{br}_{q}")
                for dx in range(3):
                    for j in range(jm):
                        nc.tensor.matmul(
                            ps[32 * j : 32 * j + 32, 0:128],
                            w3k_sb[:, (cv2 * 3 + dx) * 32 : (cv2 * 3 + dx) * 32 + 32],
                            sf2[:, 2 * (4 * q + j), dx : dx + 255 : 2],
                            start=(dx == 0),
                            stop=(dx == 2),
                            tile_position=(0, 32 * j),
                        )
                nc.scalar.activation(
                    featloc[:, q, :], ps[:, 0:128], RELU,
                    bias=conv_bias_sb[:, cv2 : cv2 + 1],
                )
            for j in range(4):
                nc.scalar.dma_start(
                    out=ag_in[br][:, j : FR : 4, :],
                    in_=featloc[32 * j : 32 * j + 32, :, :],
                )
            nc.gpsimd.collective_compute(
                "AllGather",
                mybir.AluOpType.bypass,
                replica_groups=[list(range(NCORES))],
                ins=[ag_in[br][:]],
                outs=[ag_out[br][:]],
            )
            free_featloc()
            free_sf2()
            free_feat1q()

        def proj_branch(br, spb):
            # gather full features [96, HW] (3 partition copies)
            frep, free_frep = tc.tile([96, HW], BF16, name=f"frep_{br}")
            for rep in range(3):
                src = bass.AP(
                    tensor=ag_out[br].tensor,
                    offset=ag_out[br].offset,
                    ap=[
                        [FR * FW, 32],  # ci
                        [32 * FR * FW, NCORES],  # core
                        [FW, FR],  # r
                        [1, FW],  # x
                    ],
                )
                nc.sync.dma_start(out=frep[32 * rep : 32 * rep + 32, :], in_=src)

            # k projection over full hw, 3-packed over ch chunks
            nch = 0
            rnd = 0
            while nch < 16:
                take = min(3, 16 - nch)
                ps = sp_tile(spb + rnd, f"kps_{br}_{rnd}")
                for t in range(take):
                    ch = nch + t
                    nc.tensor.matmul(
                        ps[0:32, 512 * t : 512 * t + 512],
                        wkq_sb[32 * t : 32 * t + 32, br * 64 : br * 64 + 32],
                        frep[32 * t : 32 * t + 32, 512 * ch : 512 * ch + 512],
                        start=True,
                        stop=True,
                        tile_position=(32 * t, 0),
                    )
                n = 512 * take
                nc.scalar.activation(
                    krep[br][0:32, 512 * nch : 512 * nch + n], ps[0:32, 0:n], IDENT,
                    bias=bias_kq_sb[:, br * 2 : br * 2 + 1],
                )
                nch += take
                rnd += 1
            # replicate K to partition copies, chunked for early scores
            for rep in range(1, 3):
                for c0 in range(0, HW, 2048):
                    nc.sync.dma_start(
                        out=krep[br][32 * rep : 32 * rep + 32, c0 : c0 + 2048],
                        in_=krep[br][0:32, c0 : c0 + 2048],
                    )

            # q projection over local 1024
            qrhs = small.tile([32, ILOC], BF16, name="qrhs", tag="qrhs", bufs=2)
            nc.sync.dma_start(out=qrhs[:], in_=ag_in[br][:])
            ps = sp_tile(spb + rnd, f"qps_{br}")
            for t in range(2):
                nc.tensor.matmul(
                    ps[0:32, 512 * t : 512 * t + 512],
                    wkq_sb[0:32, br * 64 + 32 : br * 64 + 64],
                    qrhs[:, 512 * t : 512 * t + 512],
                    start=True,
                    stop=True,
                    tile_position=(0, 0),
                )
            nc.scalar.activation(
                qrep[br][0:32, :], ps[0:32, 0:1024], IDENT,
                bias=bias_kq_sb[:, br * 2 + 1 : br * 2 + 2],
            )
            for rep in range(1, 3):
                nc.sync.dma_start(
                    out=qrep[br][32 * rep : 32 * rep + 32, :], in_=qrep[br][0:32, :]
                )

            # V^T blocks [128, 33] per jb (col 32 stays 1.0 from memset)
            vtv = vt[br][:].rearrange("p (b c) -> p b c", c=33)
            for gi, g0 in enumerate(range(0, NJB, GJB)):
                jbs = list(range(g0, min(g0 + GJB, NJB)))
                ps = sp_tile(spb + rnd + 1 + gi, f"vtps_{br}_{g0}")
                for t, jb in enumerate(jbs):
                    nc.tensor.matmul(
                        ps[:, 512 * t : 512 * t + 32],
                        frep[32 * t : 32 * t + 32, 128 * jb : 128 * jb + 128],
                        wvT_sb[32 * t : 32 * t + 32, br * 32 : br * 32 + 32],
                        start=True,
                        stop=True,
                        tile_position=(32 * t, 0),
                    )
                psv = ps[:].rearrange("p (t n) -> p t n", n=512)
                nc.vector.tensor_copy(
                    vtv[:, jbs[0] : jbs[0] + len(jbs), 0:32], psv[:, 0 : len(jbs), 0:32]
                )
            free_frep()

        emit_shift3(shift3_x, 1, nc.sync)
        emit_shift3(shift3_m, 0, nc.scalar)
        conv_branch(1, shift3_x, 0)
        free_shift3_x()
        conv_branch(0, shift3_m, 1)

        # ================= residual conv1 (needs shift3_m) =================
        if True:
            for q, jm in _quads(R1R):
                ps = sp_tile(q, f"r1ps_{q}")
                for dx in range(3):
                    for j in range(jm):
                        nc.tensor.matmul(
                            ps[32 * j : 32 * j + 32, 0:512],
                            w3k_sb[:, (4 * 3 + dx) * 32 : (4 * 3 + dx) * 32 + 32],
                            shift3_m[:, 4 * q + j + 1, dx : dx + 512],
                            start=(dx == 0),
                            stop=(dx == 2),
                            tile_position=(0, 32 * j),
                        )
                pm = 32 * jm
                ev = small.tile([128, 512], F32, tag="ev2")
                nc.vector.tensor_scalar(
                    ev[0:pm, :], ps[0:pm, 0:512],
                    conv_bias_sb[0:pm, 4:5], 0.0, ADD, MAX,
                )
                nc.vector.tensor_scalar(
                    resid1q[0:pm, q, 1:513], ev[0:pm, :],
                    masks_sb[0:pm, 5 + q : 6 + q], None, MULT,
                )

            free_shift3_m()
            sr2, free_sr2 = tc.tile([96, 33, BANDW], BF16, name="sr2")
            for dy in range(3):
                for jj in range(4):
                    qs = [
                        q
                        for q, jmq in _quads(R1R)
                        if jj < jmq and dy <= 4 * q + jj < dy + 33
                    ]
                    if not qs:
                        continue
                    q0, q1 = qs[0], qs[-1] + 1
                    r0 = 4 * q0 + jj - dy
                    r1 = r0 + 4 * (q1 - q0 - 1) + 1
                    nc.gpsimd.dma_start(
                        out=sr2[32 * dy : 32 * dy + 32, r0:r1:4, :],
                        in_=resid1q[32 * jj : 32 * jj + 32, q0:q1, :],
                    )

            # ---- projections x (needs AG x; runs while resid DMAs fly) ----
            proj_branch(1, 9)

            # ---- resid conv2 -> resid_sum[0]; xr 1x1 -> resid_sum[1] ----
            for q in range(FR):
                ps = sp_tile(q, f"r2ps_{q}")
                for dx in range(3):
                    for j in range(4):
                        nc.tensor.matmul(
                            ps[32 * j : 32 * j + 32, 0:512],
                            w3k_sb[:, (5 * 3 + dx) * 32 : (5 * 3 + dx) * 32 + 32],
                            sr2[:, 4 * q + j, dx : dx + 512],
                            start=(dx == 0),
                            stop=(dx == 2),
                            tile_position=(0, 32 * j),
                        )
                nc.vector.tensor_scalar(
                    resid_sum[0][:, q, :], ps[:, 0:512],
                    conv_bias_sb[:, 5:6], 0.0, ADD, MAX,
                )
                xrhs = small.tile([32, 4, 512], BF16, tag="xrhs", bufs=2)
                nc.sync.dma_start(
                    out=xrhs[:], in_=band[1][:, 4 * q + 3 : 4 * q + 7, 1:513]
                )
                ps2 = sp_tile(q + 1, f"xps_{q}")
                for j in range(4):
                    nc.tensor.matmul(
                        ps2[32 * j : 32 * j + 32, 0:512],
                        wxr_sb[:],
                        xrhs[:, j, :],
                        start=True,
                        stop=True,
                        tile_position=(0, 32 * j),
                    )
                nc.vector.tensor_scalar(
                    resid_sum[1][:, q, :], ps2[:, 0:512],
                    bias_xr_sb[:, 0:1], 0.0, ADD, MAX,
                )
            free_sr2()
            free_resid1q()

        # ---- projections m (needs AG m) ----
        proj_branch(0, 0)

        # ================= attention =================
        # av accumulators: rows 0-32 stream m (mono scores x multi V),
        # rows 64-96 stream x; cols 512*ich. Row 32/96 hold softmax
        # denominators via the ones column in vt.
        av = psum.tile([128, 1024], F32, name="av", tag="av")
        groups = []
        for g0 in range(0, NJB, GJB):
            jbs = list(range(g0, min(g0 + GJB, NJB)))
            for ich in range(2):
                groups.append((g0, ich, jbs))

        def emit_scores(slot):
            g0, ich, jbs = groups[slot]
            ex = {}
            for br in range(2):
                sp = psum.tile(
                    [128, 1536], F32, name=f"sp_{br}_{g0}_{ich}", tag=SPTAG[br]
                )
                for t, jb in enumerate(jbs):
                    nc.tensor.matmul(
                        sp[:, 512 * t : 512 * t + 512],
                        krep[br][32 * t : 32 * t + 32, 128 * jb : 128 * jb + 128],
                        qrep[br][32 * t : 32 * t + 32, 512 * ich : 512 * ich + 512],
                        start=True,
                        stop=True,
                        tile_position=(32 * t, 0),
                    )
                e = small.tile(
                    [128, 1536], BF16, name=f"ex_{br}_{g0}_{ich}", tag=f"exp{br}",
                    bufs=3,
                )
                n = 512 * len(jbs)
                nc.scalar.activation(e[:, 0:n], sp[:, 0:n], EXP)
                ex[br] = e
            return ex

        def emit_av(slot, ex):
            g0, ich, jbs = groups[slot]
            for t, jb in enumerate(jbs):
                # stream m (mono scores) x multi values -> out half 0
                nc.tensor.matmul(
                    av[0:33, 512 * ich : 512 * ich + 512],
                    vt[1][:, 33 * jb : 33 * jb + 33],
                    ex[0][:, 512 * t : 512 * t + 512],
                    start=(jb == 0),
                    stop=(jb == NJB - 1),
                    tile_position=(0, 0),
                    skip_group_check=True,
                )
                # stream x (multi scores) x mono values -> out half 1
                nc.tensor.matmul(
                    av[64:97, 512 * ich : 512 * ich + 512],
                    vt[0][:, 33 * jb : 33 * jb + 33],
                    ex[1][:, 512 * t : 512 * t + 512],
                    start=(jb == 0),
                    stop=(jb == NJB - 1),
                    tile_position=(0, 64),
                    skip_group_check=True,
                )

        pending = None
        for slot in range(len(groups)):
            ex = emit_scores(slot)
            if pending is not None:
                emit_av(*pending)
            pending = (slot, ex)
        emit_av(*pending)

        # ================= tail =================
        tailp = ctx.enter_context(tc.tile_pool(name="tailp", bufs=1))
        # denominators: av rows 32 (stream m) and 96 (stream x), 1024 wide.
        den_sb = tailp.tile([128, 1024], F32, name="den_sb")
        nc.scalar.copy(den_sb[32:33, :], av[32:33, :])
        nc.scalar.copy(den_sb[96:97, :], av[96:97, :])
        nc.sync.dma_start(out=r1d[0, :], in_=den_sb[32:33, :])
        nc.sync.dma_start(out=r1d[1, :], in_=den_sb[96:97, :])
        rcp = tailp.tile([128, 16], F32, name="rcp")
        src = bass.AP(tensor=r1d.tensor, offset=r1d.offset, ap=[[16, 128], [1, 16]])
        nc.sync.dma_start(out=rcp[:], in_=src)
        nc.vector.reciprocal(rcp[:], rcp[:])
        nc.vector.tensor_scalar(rcp[:], rcp[:], gamma_sb[:, 0:1], None, MULT)
        dst = bass.AP(tensor=r2d.tensor, offset=r2d.offset, ap=[[16, 128], [1, 16]])
        nc.sync.dma_start(out=dst, in_=rcp[:])

        for si in range(2):
            base = 64 * si
            rb = tailp.tile([32, ILOC], F32, name=f"rb_{si}", tag="rb", bufs=2)
            src = bass.AP(
                tensor=r2d.tensor,
                offset=r2d.offset + si * 1024,
                ap=[[0, 32], [1, 1024]],
            )
            nc.sync.dma_start(out=rb[:], in_=src)
            t1 = tailp.tile([32, ILOC], F32, name=f"t1_{si}", tag="t1", bufs=2)
            nc.vector.tensor_tensor(t1[:], av[base : base + 32, :], rb[:], MULT)
            nc.scalar.activation(
                attnrep[si][0:32, :], t1[:], IDENT, bias=bvg_sb[:, si : si + 1]
            )
            for rep in range(1, 4):
                nc.sync.dma_start(
                    out=attnrep[si][32 * rep : 32 * rep + 32, :],
                    in_=attnrep[si][0:32, :],
                )

        # final adds into f32 staging (2 quads per chunk), then stream out
        for si in range(2):
            for c in range(FR // 2):
                outst = tailp.tile(
                    [128, 2, 512], F32, name=f"outst_{si}_{c}", tag="outst", bufs=2
                )
                for k in range(2):
                    q = 2 * c + k
                    nc.vector.tensor_tensor(
                        outst[:, k, :].rearrange("p (a b) -> p a b", b=4),
                        resid_sum[si][:, q, :].rearrange("p (a b) -> p a b", b=4),
                        attnrep[si][:, 128 * q : 128 * q + 128, None].to_broadcast(
                            [128, 128, 4]
                        ),
                        ADD,
                    )
                for j in range(4):
                    r0 = 8 * c + j
                    nc.sync.dma_start(
                        out=out_d[32 * si : 32 * si + 32, r0 : r0 + 5 : 4, :],
                        in_=outst[32 * j : 32 * j + 32, :, :],
                    )


def kernel(**inputs):
    in_maps = _prep(inputs)
    if "nc" not in _CACHE:
        _CACHE["nc"] = build()
    res = run_bass_kernel_spmd(_CACHE["nc"], in_maps, list(range(NCORES)))
    out = np.concatenate([res.results[c]["out"] for c in range(NCORES)], axis=1)
    return out[None].astype(np.float32)


# revision 19
# speedup vs baseline: 1.0812x; 1.0507x over previous
"""CrossCueFusion Trainium2 kernel (8 NeuronCores, SPMD via bass/Tile).

Sharding: core c owns output rows [32c, 32c+32) of the [1,64,256,512]
output, feature rows [8c, 8c+8) of the 64x128 feature map (= attention
query positions [1024c, 1024c+1024)). Features are computed 1/8 per
core and AllGather'd so every core has full K / V for the global
attention; scores are computed transposed (S^T[j,i], j on partitions)
so softmax denominators come free from a ones-column in the AV matmul.

v2 schedule is exp-stream-centric: the softmax exp (16.8M elem/core on
the scalar engine, ~1 elem/lane/cycle) is the hard floor, so branch x
is processed first (its V feeds stream-m AV), projections run right
behind the AllGathers, residual convs run before attention, and the
attention loop pipelines the tensor queue one slot behind the exp
stream (scores g, then AV g-1) so neither engine waits on the other.
"""

import sys

for p in ("/opt/trn_rl_repo", "/opt/trn_rl_repo/concourse"):
    if p not in sys.path:
        sys.path.insert(0, p)

import contextlib

import ml_dtypes
import numpy as np

import concourse.bass as bass
import concourse.mybir as mybir
import concourse.tile as tile
from concourse import bacc
from concourse.bass_utils import run_bass_kernel_spmd

F32 = mybir.dt.float32
BF16 = mybir.dt.bfloat16
BF = ml_dtypes.bfloat16
EXP = mybir.ActivationFunctionType.Exp
RELU = mybir.ActivationFunctionType.Relu
IDENT = mybir.ActivationFunctionType.Identity
COPY = mybir.ActivationFunctionType.Copy
ADD = mybir.AluOpType.add
MAX = mybir.AluOpType.max
MULT = mybir.AluOpType.mult

NCORES = 8
H, W = 256, 512
FH, FW = 64, 128  # feature map
HW = FH * FW  # 8192
NJB = HW // 128  # 64 j-blocks
FR = 8  # feature rows per core
ILOC = FR * FW  # 1024 query positions per core
OUTR = 32  # output rows per core
BANDR, BANDW = 37, W + 2  # input band: rows [32c-3, 32c+34), padded width
F1R = 18  # feature conv1 rows per core: abs [16c-1, 16c+17)
R1R = 34  # resid conv1 rows per core: abs [32c-1, 32c+33)
GJB = 3  # score j-blocks per exp group

_CACHE = {}


def _quads(nrows):
    out = []
    for q in range((nrows + 3) // 4):
        out.append((q, min(4, nrows - 4 * q)))
    return out


def _prep(inputs):
    mono = np.asarray(inputs["mono_pseudo_cost"])[0]
    cost = np.asarray(inputs["cost_volume"])[0]
    g = float(np.asarray(inputs["gamma"]).reshape(-1)[0])

    def band(img, c):
        b = np.zeros((32, BANDR, BANDW), np.float32)
        r0 = 32 * c - 3
        lo, hi = max(0, r0), min(H, r0 + BANDR)
        b[:, lo - r0 : hi - r0, 1:513] = img[:, lo:hi, :]
        return b.astype(BF)

    w3k = np.zeros((96, 6, 3, 32), np.float32)
    conv_bias = np.zeros((128, 6), np.float32)
    names = [
        ("me_w1", "me_b1"),
        ("xe_w1", "xe_b1"),
        ("me_w2", "me_b2"),
        ("xe_w2", "xe_b2"),
        ("mr_w1", "mr_b1"),
        ("mr_w2", "mr_b2"),
    ]
    for cv, (wn, bn) in enumerate(names):
        w3 = np.asarray(inputs[wn])  # [o, ci, dy, dx]
        for dy in range(3):
            # [ci, dx, o]
            w3k[32 * dy : 32 * dy + 32, cv] = np.transpose(w3[:, :, dy, :], (1, 2, 0))
        conv_bias[:, cv] = np.tile(np.asarray(inputs[bn]), 4)
    w3k = w3k.reshape(96, 6 * 3 * 32).astype(BF)

    wkq32 = np.zeros((32, 128), np.float32)
    bias_kq = np.zeros((32, 4), np.float32)
    for br, (kw, kb, qw, qb) in enumerate(
        [("mk_w", "mk_b", "mq_w", "mq_b"), ("xk_w", "xk_b", "xq_w", "xq_b")]
    ):
        wkq32[:, br * 64 : br * 64 + 32] = np.asarray(inputs[kw]).T
        wkq32[:, br * 64 + 32 : br * 64 + 64] = np.asarray(inputs[qw]).T
        bias_kq[:, br * 2] = np.asarray(inputs[kb])
        bias_kq[:, br * 2 + 1] = np.asarray(inputs[qb])
    wkq = np.tile(wkq32, (3, 1)).astype(BF)  # [96, 128]: 3 partition copies

    wvT = np.zeros((96, 64), np.float32)
    for br, vw in enumerate(["mv_w", "xv_w"]):
        t = np.asarray(inputs[vw]).T  # [ci, c]
        for rep in range(3):
            wvT[32 * rep : 32 * rep + 32, br * 32 : br * 32 + 32] = t
    wvT = wvT.astype(BF)

    # stream m output (multi_out) uses multi values (xv); stream x uses mv
    bvg = np.stack(
        [g * np.asarray(inputs["xv_b"]), g * np.asarray(inputs["mv_b"])], axis=1
    ).astype(np.float32)  # [32, 2]

    wxr = np.asarray(inputs["xr_w"]).T.astype(BF)  # [ci, o]
    bias_xr = np.tile(np.asarray(inputs["xr_b"]), 4).reshape(128, 1).astype(np.float32)
    gamma_arr = np.full((128, 1), g, np.float32)

    in_maps = []
    for c in range(NCORES):
        masks = np.zeros((128, 14), np.float32)
        for q in range(5):  # feat1 quads
            for j in range(4):
                r = 16 * c - 1 + 4 * q + j
                masks[32 * j : 32 * j + 32, q] = 1.0 if 0 <= r < 128 else 0.0
        for q in range(9):  # resid1 quads
            for j in range(4):
                r = 32 * c - 1 + 4 * q + j
                masks[32 * j : 32 * j + 32, 5 + q] = 1.0 if 0 <= r < H else 0.0
        in_maps.append(
            {
                "band_m": band(mono, c),
                "band_x": band(cost, c),
                "w3k": w3k,
                "conv_bias": conv_bias,
                "masks": masks,
                "wkq": wkq,
                "bias_kq": bias_kq,
                "wvT": wvT,
                "bvg": bvg,
                "wxr": wxr,
                "bias_xr": bias_xr,
                "gamma_in": gamma_arr,
            }
        )
    return in_maps


def build():
    nc = bacc.Bacc(None)
    band_m = nc.declare_dram_parameter("band_m", [32, BANDR, BANDW], BF16, False)
    band_x = nc.declare_dram_parameter("band_x", [32, BANDR, BANDW], BF16, False)
    w3k_d = nc.declare_dram_parameter("w3k", [96, 576], BF16, False)
    conv_bias_d = nc.declare_dram_parameter("conv_bias", [128, 6], F32, False)
    masks_d = nc.declare_dram_parameter("masks", [128, 14], F32, False)
    wkq_d = nc.declare_dram_parameter("wkq", [96, 128], BF16, False)
    bias_kq_d = nc.declare_dram_parameter("bias_kq", [32, 4], F32, False)
    wvT_d = nc.declare_dram_parameter("wvT", [96, 64], BF16, False)
    bvg_d = nc.declare_dram_parameter("bvg", [32, 2], F32, False)
    wxr_d = nc.declare_dram_parameter("wxr", [32, 32], BF16, False)
    bias_xr_d = nc.declare_dram_parameter("bias_xr", [128, 1], F32, False)
    gamma_d = nc.declare_dram_parameter("gamma_in", [128, 1], F32, False)
    out_d = nc.declare_dram_parameter("out", [64, OUTR, W], F32, True)

    with tile.TileContext(nc) as tc:
        _emit(nc, tc, locals())
    nc.finalize()
    return nc


def _emit(nc, tc, d):
    band = {0: d["band_m"], 1: d["band_x"]}
    w3k_d, conv_bias_d, masks_d = d["w3k_d"], d["conv_bias_d"], d["masks_d"]
    wkq_d, bias_kq_d, wvT_d = d["wkq_d"], d["bias_kq_d"], d["wvT_d"]
    bvg_d, wxr_d, bias_xr_d, gamma_d = (
        d["bvg_d"],
        d["wxr_d"],
        d["bias_xr_d"],
        d["gamma_d"],
    )
    out_d = d["out_d"]

    ctx = contextlib.ExitStack()
    with ctx:
        persist = ctx.enter_context(tc.tile_pool(name="persist", bufs=1))
        dram = ctx.enter_context(tc.tile_pool(name="dram", bufs=1, space="DRAM"))
        psum = ctx.enter_context(tc.tile_pool(name="psum", bufs=1, space="PSUM"))
        small = ctx.enter_context(tc.tile_pool(name="small", bufs=2))

        # PSUM budget (8 banks): sp_m [128,1536] + sp_x [128,1536] + av
        # [128,1024] = 3+3+2. All conv/resid/proj/V^T psum reuses the sp
        # rings via tags so attention needs no extra banks.
        SPTAG = ("spm", "spx")

        def sp_tile(i, name):
            return psum.tile([128, 1536], F32, name=name, tag=SPTAG[i % 2])

        # ---- persistent tiles ----
        w3k_sb = persist.tile([96, 576], BF16)
        conv_bias_sb = persist.tile([128, 6], F32)
        masks_sb = persist.tile([128, 14], F32)
        wkq_sb = persist.tile([96, 128], BF16)
        bias_kq_sb = persist.tile([32, 4], F32)
        wvT_sb = persist.tile([96, 64], BF16)
        bvg_sb = persist.tile([32, 2], F32)
        wxr_sb = persist.tile([32, 32], BF16)
        bias_xr_sb = persist.tile([128, 1], F32)
        gamma_sb = persist.tile([128, 1], F32)
        krep = {
            0: persist.tile([96, HW], BF16, name="krep_m"),
            1: persist.tile([96, HW], BF16, name="krep_x"),
        }
        qrep = {
            0: persist.tile([96, ILOC], BF16, name="qrep_m"),
            1: persist.tile([96, ILOC], BF16, name="qrep_x"),
        }
        vt = {
            0: persist.tile([128, NJB * 33], BF16, name="vt_m"),
            1: persist.tile([128, NJB * 33], BF16, name="vt_x"),
        }
        attnrep = {
            0: persist.tile([128, ILOC], F32, name="attnrep_m"),
            1: persist.tile([128, ILOC], F32, name="attnrep_x"),
        }
        resid_sum = {
            0: persist.tile([128, FR, 512], BF16, name="resid_sum_m"),
            1: persist.tile([128, FR, 512], BF16, name="resid_sum_x"),
        }

        for dst, src in [
            (w3k_sb, w3k_d),
            (conv_bias_sb, conv_bias_d),
            (masks_sb, masks_d),
            (wkq_sb, wkq_d),
            (bias_kq_sb, bias_kq_d),
            (wvT_sb, wvT_d),
            (bvg_sb, bvg_d),
            (wxr_sb, wxr_d),
            (bias_xr_sb, bias_xr_d),
            (gamma_sb, gamma_d),
        ]:
            nc.sync.dma_start(out=dst[:], in_=src[:])

        ag_in = {
            0: dram.tile([32, FR, FW], BF16, name="ag_in_m"),
            1: dram.tile([32, FR, FW], BF16, name="ag_in_x"),
        }
        ag_out = {
            0: dram.tile([NCORES, 32, FR, FW], BF16, addr_space="Shared", name="ag_out_m"),
            1: dram.tile([NCORES, 32, FR, FW], BF16, addr_space="Shared", name="ag_out_x"),
        }
        r1d = dram.tile([2, 1024], F32)
        r2d = dram.tile([2, 1024], F32)

        nc.vector.memset(vt[0][:], 1.0)
        nc.vector.memset(vt[1][:], 1.0)

        # ================= feature extraction =================
        # branch x first: its V (vt[1]) feeds stream-m AV, and the m
        # projections (whose K/Q gate the first exp) then finish last.
        # Big phase buffers are manually-freed single tiles (tc.tile);
        # frees must pop in LIFO order, so resid1q (longest-lived) is
        # allocated first.
        resid1q, free_resid1q = tc.tile([128, 9, BANDW], BF16, name="resid1q")
        nc.vector.memset(resid1q[:], 0.0)
        shift3_m, free_shift3_m = tc.tile([96, BANDR, BANDW], BF16, name="shift3_m")
        shift3_x, free_shift3_x = tc.tile([96, BANDR, BANDW], BF16, name="shift3_x")

        def emit_shift3(s3, br, eng, eng2):
            # HBM load once into partitions 0-31 (dy=0 view), then build
            # the dy=1,2 partition copies with on-chip SBUF->SBUF DMAs.
            # Row-chunked so conv1 can start on the first chunk.
            for r0, r1 in ((0, 13), (13, 25), (25, BANDR)):
                eng.dma_start(
                    out=s3[0:32, r0:r1, :], in_=band[br][:, r0:r1, :]
                )
            for dy in range(1, 3):
                for r0, r1 in ((0, 13), (13, 25), (25, BANDR - dy)):
                    eng2.dma_start(
                        out=s3[32 * dy : 32 * dy + 32, r0:r1, :],
                        in_=s3[0:32, r0 + dy : r1 + dy, :],
                    )

        def conv_branch(br, s3, spb):
            cv1, cv2 = br, 2 + br
            feat1q, free_feat1q = tc.tile([128, 5, 258], BF16, name=f"feat1q_{br}")
            nc.vector.memset(feat1q[:], 0.0)
            for q, jm in _quads(F1R):
                ps = sp_tile(spb + q, f"f1ps_{br}_{q}")
                for dx in range(3):
                    for j in range(jm):
                        nc.tensor.matmul(
                            ps[32 * j : 32 * j + 32, 0:256],
                            w3k_sb[:, (cv1 * 3 + dx) * 32 : (cv1 * 3 + dx) * 32 + 32],
                            s3[:, 2 * (4 * q + j), dx : dx + 511 : 2],
                            start=(dx == 0),
                            stop=(dx == 2),
                            tile_position=(0, 32 * j),
                        )
                pm = 32 * jm
                ev = small.tile([128, 256], F32, tag="ev")
                nc.vector.tensor_scalar(
                    ev[0:pm, :], ps[0:pm, 0:256],
                    conv_bias_sb[0:pm, cv1 : cv1 + 1], 0.0, ADD, MAX,
                )
                nc.vector.tensor_scalar(
                    feat1q[0:pm, q, 1:257], ev[0:pm, :],
                    masks_sb[0:pm, q : q + 1], None, MULT,
                )

            # conv2 input shifts [96, 17, 258]
            sf2, free_sf2 = tc.tile([96, 17, 258], BF16, name=f"sf2_{br}")
            for dy in range(3):
                for jj in range(4):
                    qs = [
                        q
                        for q, jmq in _quads(F1R)
                        if jj < jmq and dy <= 4 * q + jj < dy + 17
                    ]
                    if not qs:
                        continue
                    q0, q1 = qs[0], qs[-1] + 1
                    r0 = 4 * q0 + jj - dy
                    r1 = r0 + 4 * (q1 - q0 - 1) + 1
                    nc.gpsimd.dma_start(
                        out=sf2[32 * dy : 32 * dy + 32, r0:r1:4, :],
                        in_=feat1q[32 * jj : 32 * jj + 32, q0:q1, :],
                    )

            featloc, free_featloc = tc.tile([128, 2, FW], BF16, name=f"featloc_{br}")
            for q, jm in _quads(FR):
                ps = sp_tile(spb + 5 + q, f"f2ps_{br}_{q}")
                for dx in range(3):
                    for j in range(jm):
                        nc.tensor.matmul(
                            ps[32 * j : 32 * j + 32, 0:128],
                            w3k_sb[:, (cv2 * 3 + dx) * 32 : (cv2 * 3 + dx) * 32 + 32],
                            sf2[:, 2 * (4 * q + j), dx : dx + 255 : 2],
                            start=(dx == 0),
                            stop=(dx == 2),
                            tile_position=(0, 32 * j),
                        )
                nc.scalar.activation(
                    featloc[:, q, :], ps[:, 0:128], RELU,
                    bias=conv_bias_sb[:, cv2 : cv2 + 1],
                )
            for j in range(4):
                nc.scalar.dma_start(
                    out=ag_in[br][:, j : FR : 4, :],
                    in_=featloc[32 * j : 32 * j + 32, :, :],
                )
            nc.gpsimd.collective_compute(
                "AllGather",
                mybir.AluOpType.bypass,
                replica_groups=[list(range(NCORES))],
                ins=[ag_in[br][:]],
                outs=[ag_out[br][:]],
            )
            free_featloc()
            free_sf2()
            free_feat1q()

        def proj_branch(br, spb):
            # gather full features [96, HW] (3 partition copies)
            frep, free_frep = tc.tile([96, HW], BF16, name=f"frep_{br}")
            for rep in range(3):
                src = bass.AP(
                    tensor=ag_out[br].tensor,
                    offset=ag_out[br].offset,
                    ap=[
                        [FR * FW, 32],  # ci
                        [32 * FR * FW, NCORES],  # core
                        [FW, FR],  # r
                        [1, FW],  # x
                    ],
                )
                nc.sync.dma_start(out=frep[32 * rep : 32 * rep + 32, :], in_=src)

            # k projection over full hw, 3-packed over ch chunks
            nch = 0
            rnd = 0
            while nch < 16:
                take = min(3, 16 - nch)
                ps = sp_tile(spb + rnd, f"kps_{br}_{rnd}")
                for t in range(take):
                    ch = nch + t
                    nc.tensor.matmul(
                        ps[0:32, 512 * t : 512 * t + 512],
                        wkq_sb[32 * t : 32 * t + 32, br * 64 : br * 64 + 32],
                        frep[32 * t : 32 * t + 32, 512 * ch : 512 * ch + 512],
                        start=True,
                        stop=True,
                        tile_position=(32 * t, 0),
                    )
                n = 512 * take
                nc.scalar.activation(
                    krep[br][0:32, 512 * nch : 512 * nch + n], ps[0:32, 0:n], IDENT,
                    bias=bias_kq_sb[:, br * 2 : br * 2 + 1],
                )
                nch += take
                rnd += 1
            # replicate K to partition copies, chunked for early scores
            for rep in range(1, 3):
                for c0 in range(0, HW, 2048):
                    nc.sync.dma_start(
                        out=krep[br][32 * rep : 32 * rep + 32, c0 : c0 + 2048],
                        in_=krep[br][0:32, c0 : c0 + 2048],
                    )

            # q projection over local 1024
            qrhs = small.tile([32, ILOC], BF16, name="qrhs", tag="qrhs", bufs=2)
            nc.sync.dma_start(out=qrhs[:], in_=ag_in[br][:])
            ps = sp_tile(spb + rnd, f"qps_{br}")
            for t in range(2):
                nc.tensor.matmul(
                    ps[0:32, 512 * t : 512 * t + 512],
                    wkq_sb[0:32, br * 64 + 32 : br * 64 + 64],
                    qrhs[:, 512 * t : 512 * t + 512],
                    start=True,
                    stop=True,
                    tile_position=(0, 0),
                )
            nc.scalar.activation(
                qrep[br][0:32, :], ps[0:32, 0:1024], IDENT,
                bias=bias_kq_sb[:, br * 2 + 1 : br * 2 + 2],
            )
            for rep in range(1, 3):
                nc.sync.dma_start(
                    out=qrep[br][32 * rep : 32 * rep + 32, :], in_=qrep[br][0:32, :]
                )

            # V^T blocks [128, 33] per jb (col 32 stays 1.0 from memset)
            vtv = vt[br][:].rearrange("p (b c) -> p b c", c=33)
            for gi, g0 in enumerate(range(0, NJB, GJB)):
                jbs = list(range(g0, min(g0 + GJB, NJB)))
                ps = sp_tile(spb + rnd + 1 + gi, f"vtps_{br}_{g0}")
                for t, jb in enumerate(jbs):
                    nc.tensor.matmul(
                        ps[:, 512 * t : 512 * t + 32],
                        frep[32 * t : 32 * t + 32, 128 * jb : 128 * jb + 128],
                        wvT_sb[32 * t : 32 * t + 32, br * 32 : br * 32 + 32],
                        start=True,
                        stop=True,
                        tile_position=(32 * t, 0),
                    )
                psv = ps[:].rearrange("p (t n) -> p t n", n=512)
                nc.vector.tensor_copy(
                    vtv[:, jbs[0] : jbs[0] + len(jbs), 0:32], psv[:, 0 : len(jbs), 0:32]
                )
            free_frep()

        emit_shift3(shift3_x, 1, nc.sync, nc.sync)
        emit_shift3(shift3_m, 0, nc.scalar, nc.scalar)
        conv_branch(1, shift3_x, 0)
        free_shift3_x()
        conv_branch(0, shift3_m, 1)

        # ================= residual conv1 (needs shift3_m) =================
        if True:
            for q, jm in _quads(R1R):
                ps = sp_tile(q, f"r1ps_{q}")
                for dx in range(3):
                    for j in range(jm):
                        nc.tensor.matmul(
                            ps[32 * j : 32 * j + 32, 0:512],
                            w3k_sb[:, (4 * 3 + dx) * 32 : (4 * 3 + dx) * 32 + 32],
                            shift3_m[:, 4 * q + j + 1, dx : dx + 512],
                            start=(dx == 0),
                            stop=(dx == 2),
                            tile_position=(0, 32 * j),
                        )
                pm = 32 * jm
                ev = small.tile([128, 512], F32, tag="ev2")
                nc.vector.tensor_scalar(
                    ev[0:pm, :], ps[0:pm, 0:512],
                    conv_bias_sb[0:pm, 4:5], 0.0, ADD, MAX,
                )
                nc.vector.tensor_scalar(
                    resid1q[0:pm, q, 1:513], ev[0:pm, :],
                    masks_sb[0:pm, 5 + q : 6 + q], None, MULT,
                )

            free_shift3_m()
            sr2, free_sr2 = tc.tile([96, 33, BANDW], BF16, name="sr2")
            for dy in range(3):
                for jj in range(4):
                    qs = [
                        q
                        for q, jmq in _quads(R1R)
                        if jj < jmq and dy <= 4 * q + jj < dy + 33
                    ]
                    if not qs:
                        continue
                    q0, q1 = qs[0], qs[-1] + 1
                    r0 = 4 * q0 + jj - dy
                    r1 = r0 + 4 * (q1 - q0 - 1) + 1
                    nc.gpsimd.dma_start(
                        out=sr2[32 * dy : 32 * dy + 32, r0:r1:4, :],
                        in_=resid1q[32 * jj : 32 * jj + 32, q0:q1, :],
                    )

            # ---- projections x (needs AG x; runs while resid DMAs fly) ----
            proj_branch(1, 9)

            # ---- resid conv2 -> resid_sum[0]; xr 1x1 -> resid_sum[1] ----
            for q in range(FR):
                ps = sp_tile(q, f"r2ps_{q}")
                for dx in range(3):
                    for j in range(4):
                        nc.tensor.matmul(
                            ps[32 * j : 32 * j + 32, 0:512],
                            w3k_sb[:, (5 * 3 + dx) * 32 : (5 * 3 + dx) * 32 + 32],
                            sr2[:, 4 * q + j, dx : dx + 512],
                            start=(dx == 0),
                            stop=(dx == 2),
                            tile_position=(0, 32 * j),
                        )
                nc.vector.tensor_scalar(
                    resid_sum[0][:, q, :], ps[:, 0:512],
                    conv_bias_sb[:, 5:6], 0.0, ADD, MAX,
                )
                xrhs = small.tile([32, 4, 512], BF16, tag="xrhs", bufs=2)
                nc.sync.dma_start(
                    out=xrhs[:], in_=band[1][:, 4 * q + 3 : 4 * q + 7, 1:513]
                )
                ps2 = sp_tile(q + 1, f"xps_{q}")
                for j in range(4):
                    nc.tensor.matmul(
                        ps2[32 * j : 32 * j + 32, 0:512],
                        wxr_sb[:],
                        xrhs[:, j, :],
                        start=True,
                        stop=True,
                        tile_position=(0, 32 * j),
                    )
                nc.vector.tensor_scalar(
                    resid_sum[1][:, q, :], ps2[:, 0:512],
                    bias_xr_sb[:, 0:1], 0.0, ADD, MAX,
                )
            free_sr2()
            free_resid1q()

        # ---- projections m (needs AG m) ----
        proj_branch(0, 0)

        # ================= attention =================
        # av accumulators: rows 0-32 stream m (mono scores x multi V),
        # rows 64-96 stream x; cols 512*ich. Row 32/96 hold softmax
        # denominators via the ones column in vt.
        av = psum.tile([128, 1024], F32, name="av", tag="av")
        groups = []
        for g0 in range(0, NJB, GJB):
            jbs = list(range(g0, min(g0 + GJB, NJB)))
            for ich in range(2):
                groups.append((g0, ich, jbs))

        def emit_scores(slot):
            g0, ich, jbs = groups[slot]
            ex = {}
            for br in range(2):
                sp = psum.tile(
                    [128, 1536], F32, name=f"sp_{br}_{g0}_{ich}", tag=SPTAG[br]
                )
                for t, jb in enumerate(jbs):
                    nc.tensor.matmul(
                        sp[:, 512 * t : 512 * t + 512],
                        krep[br][32 * t : 32 * t + 32, 128 * jb : 128 * jb + 128],
                        qrep[br][32 * t : 32 * t + 32, 512 * ich : 512 * ich + 512],
                        start=True,
                        stop=True,
                        tile_position=(32 * t, 0),
                    )
                e = small.tile(
                    [128, 1536], BF16, name=f"ex_{br}_{g0}_{ich}", tag=f"exp{br}",
                    bufs=3,
                )
                n = 512 * len(jbs)
                nc.scalar.activation(e[:, 0:n], sp[:, 0:n], EXP)
                ex[br] = e
            return ex

        def emit_av(slot, ex):
            g0, ich, jbs = groups[slot]
            for t, jb in enumerate(jbs):
                # stream m (mono scores) x multi values -> out half 0
                nc.tensor.matmul(
                    av[0:33, 512 * ich : 512 * ich + 512],
                    vt[1][:, 33 * jb : 33 * jb + 33],
                    ex[0][:, 512 * t : 512 * t + 512],
                    start=(jb == 0),
                    stop=(jb == NJB - 1),
                    tile_position=(0, 0),
                    skip_group_check=True,
                )
                # stream x (multi scores) x mono values -> out half 1
                nc.tensor.matmul(
                    av[64:97, 512 * ich : 512 * ich + 512],
                    vt[0][:, 33 * jb : 33 * jb + 33],
                    ex[1][:, 512 * t : 512 * t + 512],
                    start=(jb == 0),
                    stop=(jb == NJB - 1),
                    tile_position=(0, 64),
                    skip_group_check=True,
                )

        pending = None
        for slot in range(len(groups)):
            ex = emit_scores(slot)
            if pending is not None:
                emit_av(*pending)
            pending = (slot, ex)
        emit_av(*pending)

        # ================= tail =================
        tailp = ctx.enter_context(tc.tile_pool(name="tailp", bufs=1))
        # denominators: av rows 32 (stream m) and 96 (stream x), 1024 wide.
        den_sb = tailp.tile([128, 1024], F32, name="den_sb")
        nc.scalar.copy(den_sb[32:33, :], av[32:33, :])
        nc.scalar.copy(den_sb[96:97, :], av[96:97, :])
        nc.sync.dma_start(out=r1d[0, :], in_=den_sb[32:33, :])
        nc.sync.dma_start(out=r1d[1, :], in_=den_sb[96:97, :])
        rcp = tailp.tile([128, 16], F32, name="rcp")
        src = bass.AP(tensor=r1d.tensor, offset=r1d.offset, ap=[[16, 128], [1, 16]])
        nc.sync.dma_start(out=rcp[:], in_=src)
        nc.vector.reciprocal(rcp[:], rcp[:])
        nc.vector.tensor_scalar(rcp[:], rcp[:], gamma_sb[:, 0:1], None, MULT)
        dst = bass.AP(tensor=r2d.tensor, offset=r2d.offset, ap=[[16, 128], [1, 16]])
        nc.sync.dma_start(out=dst, in_=rcp[:])

        for si in range(2):
            base = 64 * si
            rb = tailp.tile([32, ILOC], F32, name=f"rb_{si}", tag="rb", bufs=2)
            src = bass.AP(
                tensor=r2d.tensor,
                offset=r2d.offset + si * 1024,
                ap=[[0, 32], [1, 1024]],
            )
            nc.sync.dma_start(out=rb[:], in_=src)
            t1 = tailp.tile([32, ILOC], F32, name=f"t1_{si}", tag="t1", bufs=2)
            nc.vector.tensor_tensor(t1[:], av[base : base + 32, :], rb[:], MULT)
            nc.scalar.activation(
                attnrep[si][0:32, :], t1[:], IDENT, bias=bvg_sb[:, si : si + 1]
            )
            for rep in range(1, 4):
                nc.sync.dma_start(
                    out=attnrep[si][32 * rep : 32 * rep + 32, :],
                    in_=attnrep[si][0:32, :],
                )

        # final adds into f32 staging (2 quads per chunk), then stream out
        for si in range(2):
            for c in range(FR // 2):
                outst = tailp.tile(
                    [128, 2, 512], F32, name=f"outst_{si}_{c}", tag="outst", bufs=2
                )
                for k in range(2):
                    q = 2 * c + k
                    nc.vector.tensor_tensor(
                        outst[:, k, :].rearrange("p (a b) -> p a b", b=4),
                        resid_sum[si][:, q, :].rearrange("p (a b) -> p a b", b=4),
                        attnrep[si][:, 128 * q : 128 * q + 128, None].to_broadcast(
                            [128, 128, 4]
                        ),
                        ADD,
                    )
                for j in range(4):
                    r0 = 8 * c + j
                    nc.sync.dma_start(
                        out=out_d[32 * si : 32 * si + 32, r0 : r0 + 5 : 4, :],
                        in_=outst[32 * j : 32 * j + 32, :, :],
                    )


def kernel(**inputs):
    in_maps = _prep(inputs)
    if "nc" not in _CACHE:
        _CACHE["nc"] = build()
    res = run_bass_kernel_spmd(_CACHE["nc"], in_maps, list(range(NCORES)))
    out = np.concatenate([res.results[c]["out"] for c in range(NCORES)], axis=1)
    return out[None].astype(np.float32)
